# revision 1
# baseline (speedup 1.0000x reference)
"""Trainium2 Bass kernel for conv-qkv rank-1 attention.

out = gamma * q * sum(k*v) + x, where q,k,v are per-time-slice 3x3 convs
(C=64 -> C=64) of x [B=8, C=64, T=16, W=64, H=64].

Sharding: data-parallel over B across 8 cores (1 example/core), conv
weights replicated. No cross-core communication.

Per-core schedule: T slices processed in pairs; slice t lives on SBUF
partitions 0-63, slice t+1 on partitions 64-127, giving two concurrent
PE row-group chains (K=64 each). Each 3x3 conv = 9 shifted matmul taps
(+1 bias tap with an all-ones rhs) accumulated in PSUM. Stationary
[Wq|Wk] (M=128) produces q,k in one bank; Wv (M=64) is column-placed so
k and v land on the same partitions for the fused DVE k*v+reduce.
Matmuls run in float32r (FP22 truncation on read, 1 cycle/row).
"""

import numpy as np

import concourse.bacc as bacc
import concourse.bass as bass
import concourse.mybir as mybir
import concourse.tile as tile
from concourse import bass_utils

F32 = mybir.dt.float32
F32R = mybir.dt.float32r
ALU = mybir.AluOpType

B, C, T, W, H = 8, 64, 16, 64, 64
WP, HP = W + 2, H + 2          # padded slice dims
import os
NPAIR = int(os.environ.get("BASS_NPAIR", T // 2))  # slice pairs per core
RB = 8                         # W-rows per pixel block
NBLK = W // RB                 # pixel blocks per slice
BN = RB * H                    # moving free dim per matmul (512)
NTAP = 10                      # 9 conv taps + 1 bias tap


def _round22(a: np.ndarray) -> np.ndarray:
    """Round fp32 to 11 mantissa bits so the PE's FP22 read-truncation is
    exact (unbiased quantization instead of truncation)."""
    u = np.ascontiguousarray(a, np.float32).view(np.uint32).astype(np.uint64)
    u = ((u + 0x800) & 0xFFFFF000).astype(np.uint32)
    return u.view(np.float32)


def _pack_weights(wq, wk, wv, bq, bk, bv):
    """Pack stationary operands.

    wqk [128, 10, 128]: partitions 0-63 = chain-low taps ([Wq | Wk] so q
    lands on psum partitions 0-63, matching x_t's partitions), partitions
    64-127 = chain-high taps ([Wk | Wq], q on partitions 64-127). Tap 9 is
    the bias tap (row 0 = biases, used with an all-ones rhs).
    wv2 [128, 10, 64]: Wv taps for both chains (same values).
    """
    def taps(w):  # [O, I, 1, 3, 3] -> [I, 9, O]
        return np.ascontiguousarray(
            w.reshape(C, C, 9).transpose(1, 2, 0), np.float32)

    wq_t, wk_t, wv_t = taps(wq), taps(wk), taps(wv)
    # [Wk | Wq] for both chains: k lands on psum partitions 0-63 (the
    # custom DVE reduce op requires base partition 0), q on 64-127
    wqk = np.zeros((128, NTAP, 128), np.float32)
    wqk[0:64, 0:9, 0:64] = wk_t
    wqk[0:64, 0:9, 64:128] = wq_t
    wqk[64:128, 0:9, 0:64] = wk_t
    wqk[64:128, 0:9, 64:128] = wq_t
    wqk[0, 9, 0:64] = bk
    wqk[0, 9, 64:128] = bq
    wqk[64, 9, 0:64] = bk
    wqk[64, 9, 64:128] = bq

    # v stationary is [Wv | Wv] (M=128): the duplicated column half costs
    # nothing (M=64 would leave the array half idle) and lets every matmul
    # use column position 0, which fp32r codegen requires
    wv2 = np.zeros((128, NTAP, 128), np.float32)
    wv2[0:64, 0:9, 0:64] = wv_t
    wv2[0:64, 0:9, 64:128] = wv_t
    wv2[64:128, 0:9, 0:64] = wv_t
    wv2[64:128, 0:9, 64:128] = wv_t
    wv2[0, 9, 0:64] = bv
    wv2[0, 9, 64:128] = bv
    wv2[64, 9, 0:64] = bv
    wv2[64, 9, 64:128] = bv
    return _round22(wqk), _round22(wv2)


def _emit(nc, tc, x_d, wqk_d, wv_d, gam_d, ones_d, zer_d, out_d, ctx):
    const = ctx.enter_context(tc.tile_pool(name="const", bufs=1))
    state = ctx.enter_context(tc.tile_pool(name="state", bufs=1))
    psum = ctx.enter_context(
        tc.tile_pool(name="psum", bufs=2, space=bass.MemorySpace.PSUM))
    vpool = ctx.enter_context(tc.tile_pool(name="vpool", bufs=2))

    wqk_t = const.tile([128, NTAP, 128], F32R, tag="wqk")
    wv_t = const.tile([128, NTAP, 128], F32R, tag="wv")
    gam_t = const.tile([128, 1], F32, tag="gam")
    ones_t = const.tile([128, BN], F32R, tag="ones")

    nc.sync.dma_start(wqk_t[:], wqk_d[:])
    nc.sync.dma_start(wv_t[:], wv_d[:])
    nc.sync.dma_start(gam_t[:], gam_d[:])
    nc.sync.dma_start(ones_t[:], ones_d[:])

    xp = [state.tile([128, WP, HP], F32R, tag=f"xp{i}", name=f"xp{i}") for i in range(3)]
    qs = [state.tile([128, W * H], F32, tag=f"qs{i}", name=f"qs{i}") for i in range(2)]
    ot = [state.tile([128, W * H], F32, tag=f"ot{i}", name=f"ot{i}") for i in range(2)]
    scr = state.tile([128, BN], F32, tag="scr")
    sparts = [state.tile([64, 2, NBLK], F32, tag=f"sp{i}", name=f"sp{i}") for i in range(2)]
    sgam = [state.tile([64, 2], F32, tag=f"sg{i}", name=f"sg{i}") for i in range(2)]
    sfin = [state.tile([128, 1], F32, tag=f"sf{i}", name=f"sf{i}") for i in range(2)]

    # zero the padding ring of both x buffers once (gpsimd memset does not
    # take f32r, so DMA from a host-provided zero vector); interior DMAs
    # never touch the ring
    for t_ in xp:
        nc.sync.dma_start(t_[:, 0, :], zer_d[:, :])
        nc.sync.dma_start(t_[:, WP - 1, :], zer_d[:, :])
        nc.sync.dma_start(t_[:, :, 0], zer_d[:, 0:WP])
        nc.sync.dma_start(t_[:, :, HP - 1], zer_d[:, 0:WP])

    def load_pair(p):
        t_ = xp[p % 3]
        nc.sync.dma_start(t_[0:64, 1:1 + W, 1:1 + H], x_d[:, 2 * p])
        nc.sync.dma_start(t_[64:128, 1:1 + W, 1:1 + H], x_d[:, 2 * p + 1])

    load_pair(0)
    if NPAIR > 1:
        load_pair(1)

    for p in range(NPAIR):
        pb = p % 2
        xp_, qs_, ot_ = xp[p % 3], qs[pb], ot[pb]

        # prefetch two pairs ahead; emitted before this pair's s-swap DMA
        # so the serial sync queue never holds the x-load behind it
        if p + 2 < NPAIR:
            load_pair(p + 2)

        for j in range(NBLK):
            qk_lo = psum.tile([128, BN], F32, tag="qk_lo")
            qk_hi = psum.tile([128, BN], F32, tag="qk_hi")
            v_lo = psum.tile([128, BN], F32, tag="v_lo", name="v_lo")
            v_hi = psum.tile([128, BN], F32, tag="v_hi", name="v_hi")

            def rhs(half, tap):
                if tap == 9:
                    return ones_t[64 * half:64 * half + 64, :]
                dy, dx = tap // 3, tap % 3
                r0 = j * RB + dy
                return xp_[64 * half:64 * half + 64,
                           r0:r0 + RB, dx:dx + H]

            for tap in range(NTAP):
                st, sp = tap == 0, tap == NTAP - 1
                nc.tensor.matmul(
                    qk_lo[:, :],
                    wqk_t[0:64, tap, :],
                    rhs(0, tap), start=st, stop=sp)
                nc.tensor.matmul(
                    qk_hi[:, :],
                    wqk_t[64:128, tap, :],
                    rhs(1, tap), start=st, stop=sp)
            for tap in range(NTAP):
                st, sp = tap == 0, tap == NTAP - 1
                nc.tensor.matmul(
                    v_lo[:, :],
                    wv_t[0:64, tap, :],
                    rhs(0, tap), start=st, stop=sp)
                nc.tensor.matmul(
                    v_hi[:, :],
                    wv_t[64:128, tap, :],
                    rhs(1, tap), start=st, stop=sp)

            # evacuate q and v on ScalarE (DVE may read only one PSUM
            # operand, so v must reach SBUF before the fused k*v reduce).
            # q_t moves partitions 64-127 -> 0-63 to line up with x_t.
            if os.environ.get("BASS_QCROSS", "1") == "1":
                nc.scalar.copy(qs_[0:64, j * BN:(j + 1) * BN], qk_lo[64:128, :])
            else:
                nc.scalar.copy(qs_[0:64, j * BN:(j + 1) * BN], qk_lo[0:64, :])
            nc.scalar.copy(qs_[64:128, j * BN:(j + 1) * BN], qk_hi[64:128, :])
            vsb_lo = vpool.tile([64, BN], F32, tag="vsb_lo", name="vsb_lo")
            vsb_hi = vpool.tile([64, BN], F32, tag="vsb_hi", name="vsb_hi")
            nc.scalar.copy(vsb_lo[:, :], v_lo[0:64, :])
            nc.scalar.copy(vsb_hi[:, :], v_hi[0:64, :])

            # fused k*v multiply + pixel-sum (k from PSUM at base partition
            # 0 -- the custom DVE op requires it; v from SBUF)
            if os.environ.get("BASS_TTR", "1") == "1":
                # native TensorScalarPtr with accumulate: one DVE pass does
                # k*v and the pixel-sum
                nc.vector.scalar_tensor_tensor(
                    out=scr[0:64, :], in0=qk_lo[0:64, :], scalar=1.0,
                    in1=vsb_lo[:, :], op0=ALU.mult, op1=ALU.mult,
                    accum_out=sparts[pb][:, 0, j:j + 1])
                nc.vector.scalar_tensor_tensor(
                    out=scr[0:64, :], in0=qk_hi[0:64, :], scalar=1.0,
                    in1=vsb_hi[:, :], op0=ALU.mult, op1=ALU.mult,
                    accum_out=sparts[pb][:, 1, j:j + 1])
            else:
                nc.vector.tensor_tensor(
                    out=scr[0:64, :], in0=qk_lo[0:64, :], in1=vsb_lo[:, :],
                    op=ALU.mult)
                nc.vector.reduce_sum(sparts[pb][:, 0, j:j + 1], scr[0:64, :],
                                     axis=mybir.AxisListType.X)
                nc.vector.tensor_tensor(
                    out=scr[0:64, :], in0=qk_hi[0:64, :], in1=vsb_hi[:, :],
                    op=ALU.mult)
                nc.vector.reduce_sum(sparts[pb][:, 1, j:j + 1], scr[0:64, :],
                                     axis=mybir.AxisListType.X)

        nc.vector.reduce_sum(sgam[pb][:, :], sparts[pb][:, :, :],
                             axis=mybir.AxisListType.X)
        nc.vector.tensor_scalar_mul(sgam[pb][:, :], sgam[pb][:, :],
                                    gam_t[0:64, 0:1])
        # s_{t+1} is accumulated on partitions 0-63 but q_{t+1}/x_{t+1}
        # live on 64-127: move it with a tiny sbuf->sbuf DMA
        if os.environ.get("BASS_SWAPDMA", "1") == "1":
            nc.sync.dma_start(sfin[pb][64:128, :], sgam[pb][:, 1:2])
        else:
            nc.vector.tensor_copy(sfin[pb][0:64, :], sgam[pb][:, 1:2])

        for j in range(NBLK):
            # out = (q * (gamma*s)) + x, fused
            nc.vector.scalar_tensor_tensor(
                out=ot_[0:64, j * BN:(j + 1) * BN],
                in0=qs_[0:64, j * BN:(j + 1) * BN],
                scalar=sgam[pb][:, 0:1],
                in1=xp_[0:64, 1 + j * RB:1 + (j + 1) * RB, 1:1 + H].bitcast(F32),
                op0=ALU.mult, op1=ALU.add)
            nc.vector.scalar_tensor_tensor(
                out=ot_[64:128, j * BN:(j + 1) * BN],
                in0=qs_[64:128, j * BN:(j + 1) * BN],
                scalar=sfin[pb][64:128, 0:1],
                in1=xp_[64:128, 1 + j * RB:1 + (j + 1) * RB, 1:1 + H].bitcast(F32),
                op0=ALU.mult, op1=ALU.add)

        nc.gpsimd.dma_start(out_d[:, 2 * p], ot_[0:64, :])
        nc.gpsimd.dma_start(out_d[:, 2 * p + 1], ot_[64:128, :])


_ONES = np.ones((128, BN), np.float32)
_ZER = np.zeros((128, HP), np.float32)

_CACHE = {}


def _build():
    if "nc" in _CACHE:
        return _CACHE["nc"]
    nc = bacc.Bacc("TRN2", target_bir_lowering=False, debug=False,
                   enable_asserts=False, num_devices=8)
    x_d = nc.dram_tensor("x", (C, T, W, H), F32R, kind="ExternalInput").ap()
    wqk_d = nc.dram_tensor("wqk", (128, NTAP, 128), F32R,
                           kind="ExternalInput").ap()
    wv_d = nc.dram_tensor("wv2", (128, NTAP, 128), F32R,
                          kind="ExternalInput").ap()
    gam_d = nc.dram_tensor("gamma_bc", (128, 1), F32,
                           kind="ExternalInput").ap()
    ones_d = nc.dram_tensor("ones", (128, BN), F32R,
                            kind="ExternalInput").ap()
    zer_d = nc.dram_tensor("zer", (128, HP), F32R,
                           kind="ExternalInput").ap()
    out_d = nc.dram_tensor("out", (C, T, W, H), F32,
                           kind="ExternalOutput").ap()
    from contextlib import ExitStack
    with tile.TileContext(nc) as tc, ExitStack() as ctx:
        _emit(nc, tc, x_d, wqk_d, wv_d, gam_d, ones_d, zer_d, out_d, ctx)
    nc.compile()
    _CACHE["nc"] = nc
    return nc


def run_spmd(x, wq, wk, wv, bq, bk, bv, gamma, trace=False, **kw):
    nc = _build()
    wqk, wv2 = _pack_weights(
        np.asarray(wq, np.float32), np.asarray(wk, np.float32),
        np.asarray(wv, np.float32), np.asarray(bq, np.float32),
        np.asarray(bk, np.float32), np.asarray(bv, np.float32))
    gam = np.full((128, 1), np.float32(np.asarray(gamma).reshape(-1)[0]),
                  np.float32)
    x = np.asarray(x, np.float32)
    in_maps = [
        {"x": np.ascontiguousarray(x[b]), "wqk": wqk, "wv2": wv2,
         "gamma_bc": gam, "ones": _ONES, "zer": _ZER}
        for b in range(B)
    ]
    res = bass_utils.run_bass_kernel_spmd(
        nc, in_maps, core_ids=list(range(B)), trace=trace, **kw)
    out = np.stack([res.results[b]["out"] for b in range(B)], axis=0)
    return out, res


def kernel(x, wq, wk, wv, bq, bk, bv, gamma):
    out, _ = run_spmd(x, wq, wk, wv, bq, bk, bv, gamma)
    return out



# revision 11
# speedup vs baseline: 1.6377x; 1.6377x over previous
"""Trainium2 Bass kernel for conv-qkv rank-1 attention.

out = gamma * (q+bq) * sum((k+bk)*(v+bv)) + x, where q,k,v are
per-time-slice 3x3 convs (C=64 -> C=64) of x [B=8, C=64, T=16, W=64, H=64].

Sharding: data-parallel over B across 8 cores (1 example/core), conv
weights replicated. No cross-core communication.

v2 design (vs v1 baseline at ~492us):
- No bias matmul taps: bq/bv folded into the ACT/Pool evacuation bias,
  bk folded into the DVE STT op0-add scalar, gamma folded into wv/bv
  host-side. 18 PE pair-slots per (pair, block) instead of 20.
- Stationary weights in bf16: LDWEIGHTS streams half the bytes (it was
  longer than the 512-row matmul itself), moving x stays f32r.
- Host pads H to 66 so each x slice loads with ONE contiguous
  descriptor per partition (was 64 x 256B strided descriptors -> 57us
  serial startup and 205us of DMA activity).
- W-pad rows zeroed once with on-chip memzero (no zero-vector DMAs).
- Merged [128,512] out-STT (q*s+x for both slices at once), s for the
  hi slice moved 0:63 -> 64:127 with a tiny sbuf-sbuf DMA on the
  vector queue.
- v evacuated by Pool (gpsimd), q by ACT, k*v+reduce and out on DVE.
"""

import os

import numpy as np

import concourse.bacc as bacc
import concourse.bass as bass
import concourse.mybir as mybir
import concourse.tile as tile
from concourse import bass_utils

F32 = mybir.dt.float32
F32R = mybir.dt.float32r
BF16 = mybir.dt.bfloat16
ALU = mybir.AluOpType
ACTF = mybir.ActivationFunctionType

B, C, T, W, H = 8, 64, 16, 64, 64
HP = H + 2                     # host-padded H
WP = W + 2                     # SBUF-padded W rows
NPAIR = T // 2                 # slice pairs per core
RB = 8                         # W-rows per pixel block
NBLK = W // RB                 # pixel blocks per slice
BN = RB * H                    # moving free dim per matmul (512)
NTAP = 9                       # conv taps (no bias tap)

XDT_NAME = os.environ.get("BASS_XDT", "f32r")   # moving/x dtype
# walrus rejects mixed 32/16-bit matmul inputs: stationary follows moving
WDT_NAME = os.environ.get("BASS_WDT", "bf16" if XDT_NAME == "bf16" else "f32r")
# GPSIMD cannot access PSUM (BIR verifier) -> evacuations must use ACT
VEVAC = os.environ.get("BASS_VEVAC", "act")     # pool | act


def _round22(a: np.ndarray) -> np.ndarray:
    """Round fp32 to 11 mantissa bits so the PE's FP22 read-truncation is
    exact (unbiased quantization instead of truncation)."""
    u = np.ascontiguousarray(a, np.float32).view(np.uint32).astype(np.uint64)
    u = ((u + 0x800) & 0xFFFFF000).astype(np.uint32)
    return u.view(np.float32)


def _to_bf16(a: np.ndarray) -> np.ndarray:
    import ml_dtypes
    return np.ascontiguousarray(a, np.float32).astype(ml_dtypes.bfloat16)


def _pack_w(a: np.ndarray) -> np.ndarray:
    return _to_bf16(a) if WDT_NAME == "bf16" else _round22(a)


def _pack_weights(wq, wk, wv, bq, bk, bv, gamma):
    """Pack stationary operands (no bias rows; gamma folded into wv/bv).

    wkq [128, 9, 128]: [Wk | Wq] on both partition halves (k lands on
    psum partitions 0-63 for the DVE accum op, q on 64-127).
    wv2 [128, 9, 64]: gamma*Wv on both halves (M=64).
    bias [128, 3]: col0=bq, col1=bk, col2=gamma*bv, duplicated halves.
    """
    g = float(np.asarray(gamma).reshape(-1)[0])

    def taps(w):  # [O, I, 1, 3, 3] -> [I, 9, O]
        return np.ascontiguousarray(
            w.reshape(C, C, 9).transpose(1, 2, 0), np.float32)

    wq_t, wk_t, wv_t = taps(wq), taps(wk), taps(wv) * g
    wkq = np.zeros((128, NTAP, 128), np.float32)
    wkq[0:64, :, 0:64] = wk_t
    wkq[0:64, :, 64:128] = wq_t
    wkq[64:128, :, 0:64] = wk_t
    wkq[64:128, :, 64:128] = wq_t

    wv2 = np.zeros((128, NTAP, 64), np.float32)
    wv2[0:64] = wv_t
    wv2[64:128] = wv_t

    bias = np.zeros((128, 3), np.float32)
    bias[0:64, 0] = bq
    bias[64:128, 0] = bq
    bias[0:64, 1] = bk
    bias[64:128, 1] = bk
    bias[0:64, 2] = bv * g
    bias[64:128, 2] = bv * g
    return _pack_w(wkq), _pack_w(wv2), bias


def _emit(nc, tc, x_d, wkq_d, wv_d, bias_d, zer_d, out_d, ctx):
    xdt = F32R if XDT_NAME == "f32r" else BF16  # storage dtype of x tiles

    const = ctx.enter_context(tc.tile_pool(name="const", bufs=1))
    state = ctx.enter_context(tc.tile_pool(name="state", bufs=1))
    psum = ctx.enter_context(
        tc.tile_pool(name="psum", bufs=2, space=bass.MemorySpace.PSUM))
    vpool = ctx.enter_context(tc.tile_pool(name="vpool", bufs=2))

    wdt = BF16 if WDT_NAME == "bf16" else F32R
    wkq_t = const.tile([128, NTAP, 128], wdt, tag="wkq")
    wv_t = const.tile([128, NTAP, 64], wdt, tag="wv")
    bias_t = const.tile([128, 3], F32, tag="bias")

    xp = [state.tile([128, WP, HP], xdt, tag=f"xp{i}", name=f"xp{i}")
          for i in range(3)]
    qs = [state.tile([128, W * H], F32, tag=f"qs{i}", name=f"qs{i}")
          for i in range(2)]
    ot = [state.tile([128, W * H], F32, tag=f"ot{i}", name=f"ot{i}")
          for i in range(2)]
    scr = [state.tile([64, BN], F32, tag=f"scr{i}", name=f"scr{i}")
           for i in range(2)]
    sparts = [state.tile([64, 2, NBLK], F32, tag=f"sp{i}", name=f"sp{i}")
              for i in range(2)]
    sfull = [state.tile([128, 1], F32, tag=f"sf{i}", name=f"sf{i}")
             for i in range(2)]
    stmp = [state.tile([64, 1], F32, tag=f"st{i}", name=f"st{i}")
            for i in range(2)]

    # x first: the first pair's load gates the first matmul
    def load_pair(p):
        t_ = xp[p % 3]
        nc.sync.dma_start(t_[0:64, 1:1 + W, :], x_d[:, 2 * p])
        nc.sync.dma_start(t_[64:128, 1:1 + W, :], x_d[:, 2 * p + 1])

    load_pair(0)
    nc.gpsimd.dma_start(wkq_t[:], wkq_d[:])
    nc.gpsimd.dma_start(wv_t[:], wv_d[:])
    nc.gpsimd.dma_start(bias_t[:], bias_d[:])
    # zero the W-pad rows once (H-pad columns come zeroed from the host).
    # The BIR verifier rejects compute-engine writes feeding an fp32r
    # matmul, so in f32r mode the zeros come from a host tensor via DMA
    # (on the ACT queue, off the x-load path).
    for t_ in xp:
        if xdt == F32R:
            nc.scalar.dma_start(t_[:, 0, :], zer_d[:, :])
            nc.scalar.dma_start(t_[:, WP - 1, :], zer_d[:, :])
        else:
            nc.scalar.memzero(t_[:, 0, :])
            nc.scalar.memzero(t_[:, WP - 1, :])
    load_pair(1)

    def mm_rhs(xp_, half, tap, j):
        dy, dx = tap // 3, tap % 3
        r0 = j * RB + dy
        return xp_[64 * half:64 * half + 64, r0:r0 + RB, dx:dx + H]

    for p in range(NPAIR):
        pb = p % 2
        xp_, qs_, ot_ = xp[p % 3], qs[pb], ot[pb]

        if p + 2 < NPAIR:
            load_pair(p + 2)

        for j in range(NBLK):
            v_lo = psum.tile([128, BN], F32, tag="v_lo", name="v_lo")
            v_hi = psum.tile([128, BN], F32, tag="v_hi", name="v_hi")
            kq_lo = psum.tile([128, BN], F32, tag="kq_lo")
            kq_hi = psum.tile([128, BN], F32, tag="kq_hi")

            for tap in range(NTAP):
                st, sp = tap == 0, tap == NTAP - 1
                nc.tensor.matmul(
                    v_lo[0:64, :], wv_t[0:64, tap, :],
                    mm_rhs(xp_, 0, tap, j), start=st, stop=sp)
                nc.tensor.matmul(
                    v_hi[0:64, :], wv_t[64:128, tap, :],
                    mm_rhs(xp_, 1, tap, j), start=st, stop=sp)
            for tap in range(NTAP):
                st, sp = tap == 0, tap == NTAP - 1
                nc.tensor.matmul(
                    kq_lo[:, :], wkq_t[0:64, tap, :],
                    mm_rhs(xp_, 0, tap, j), start=st, stop=sp)
                nc.tensor.matmul(
                    kq_hi[:, :], wkq_t[64:128, tap, :],
                    mm_rhs(xp_, 1, tap, j), start=st, stop=sp)

            # v + bv -> SBUF (Pool), q + bq -> SBUF (ACT; lo crosses
            # partitions 64-127 -> 0-63 to line up with x_t)
            vsb_lo = vpool.tile([64, BN], F32, tag="vsb_lo", name="vsb_lo")
            vsb_hi = vpool.tile([64, BN], F32, tag="vsb_hi", name="vsb_hi")
            if VEVAC == "pool":
                nc.gpsimd.tensor_scalar_add(
                    vsb_lo[:, :], v_lo[0:64, :], bias_t[0:64, 2:3])
                nc.gpsimd.tensor_scalar_add(
                    vsb_hi[:, :], v_hi[0:64, :], bias_t[0:64, 2:3])
            else:
                nc.scalar.activation(
                    vsb_lo[:, :], v_lo[0:64, :], ACTF.Identity,
                    bias=bias_t[0:64, 2:3])
                nc.scalar.activation(
                    vsb_hi[:, :], v_hi[0:64, :], ACTF.Identity,
                    bias=bias_t[0:64, 2:3])
            nc.scalar.activation(
                qs_[0:64, j * BN:(j + 1) * BN], kq_lo[64:128, :],
                ACTF.Identity, bias=bias_t[64:128, 0:1])
            nc.scalar.activation(
                qs_[64:128, j * BN:(j + 1) * BN], kq_hi[64:128, :],
                ACTF.Identity, bias=bias_t[64:128, 0:1])

            # (k+bk)*v with pixel-sum accumulation (k from PSUM base 0)
            nc.vector.scalar_tensor_tensor(
                out=scr[0][:, :], in0=kq_lo[0:64, :],
                scalar=bias_t[0:64, 1:2], in1=vsb_lo[:, :],
                op0=ALU.add, op1=ALU.mult,
                accum_out=sparts[pb][:, 0, j:j + 1])
            nc.vector.scalar_tensor_tensor(
                out=scr[1][:, :], in0=kq_hi[0:64, :],
                scalar=bias_t[0:64, 1:2], in1=vsb_hi[:, :],
                op0=ALU.add, op1=ALU.mult,
                accum_out=sparts[pb][:, 1, j:j + 1])

        nc.vector.tensor_reduce(
            sfull[pb][0:64, 0:1], sparts[pb][:, 0, :],
            axis=mybir.AxisListType.X, op=ALU.add)
        nc.vector.tensor_reduce(
            stmp[pb][:, 0:1], sparts[pb][:, 1, :],
            axis=mybir.AxisListType.X, op=ALU.add)
        # s_{t+1} accumulates on partitions 0-63 but q_{t+1}/x_{t+1} live
        # on 64-127: move it with a tiny sbuf->sbuf DMA on the ACT queue
        nc.scalar.dma_start(sfull[pb][64:128, :], stmp[pb][:, :])

        for j in range(NBLK):
            in1 = xp_[:, 1 + j * RB:1 + (j + 1) * RB, 1:1 + H]
            if xdt == F32R:
                in1 = in1.bitcast(F32)
            nc.vector.scalar_tensor_tensor(
                out=ot_[:, j * BN:(j + 1) * BN],
                in0=qs_[:, j * BN:(j + 1) * BN],
                scalar=sfull[pb][:, 0:1],
                in1=in1,
                op0=ALU.mult, op1=ALU.add)

        nc.gpsimd.dma_start(out_d[:, 2 * p], ot_[0:64, :])
        nc.gpsimd.dma_start(out_d[:, 2 * p + 1], ot_[64:128, :])


_CACHE = {}


def _build():
    key = (XDT_NAME, WDT_NAME, VEVAC)
    if key in _CACHE:
        return _CACHE[key]
    nc = bacc.Bacc("TRN2", target_bir_lowering=False, debug=False,
                   enable_asserts=False, num_devices=8)
    xdt = F32R if XDT_NAME == "f32r" else BF16
    wdt = BF16 if WDT_NAME == "bf16" else F32R
    x_d = nc.dram_tensor("xpad", (C, T, W, HP), xdt,
                         kind="ExternalInput").ap()
    wkq_d = nc.dram_tensor("wkq", (128, NTAP, 128), wdt,
                           kind="ExternalInput").ap()
    wv_d = nc.dram_tensor("wv2", (128, NTAP, 64), wdt,
                          kind="ExternalInput").ap()
    bias_d = nc.dram_tensor("biases", (128, 3), F32,
                            kind="ExternalInput").ap()
    zer_d = nc.dram_tensor("zer", (128, HP), xdt,
                           kind="ExternalInput").ap()
    out_d = nc.dram_tensor("out", (C, T, W, H), F32,
                           kind="ExternalOutput").ap()
    from contextlib import ExitStack
    with tile.TileContext(nc) as tc, ExitStack() as ctx:
        _emit(nc, tc, x_d, wkq_d, wv_d, bias_d, zer_d, out_d, ctx)
    nc.compile()
    _CACHE[key] = nc
    return nc


def run_spmd(x, wq, wk, wv, bq, bk, bv, gamma, trace=False, **kw):
    nc = _build()
    wkq, wv2, biases = _pack_weights(
        np.asarray(wq, np.float32), np.asarray(wk, np.float32),
        np.asarray(wv, np.float32), np.asarray(bq, np.float32),
        np.asarray(bk, np.float32), np.asarray(bv, np.float32),
        np.asarray(gamma, np.float32))
    x = np.asarray(x, np.float32)
    xpad = np.zeros((B, C, T, W, HP), np.float32)
    xpad[..., 1:1 + H] = x
    zer = np.zeros((128, HP), np.float32)
    if XDT_NAME == "bf16":
        xpad = _to_bf16(xpad)
        zer = _to_bf16(zer)
    in_maps = [
        {"xpad": np.ascontiguousarray(xpad[b]), "wkq": wkq, "wv2": wv2,
         "biases": biases, "zer": zer}
        for b in range(B)
    ]
    res = bass_utils.run_bass_kernel_spmd(
        nc, in_maps, core_ids=list(range(B)), trace=trace, **kw)
    out = np.stack([res.results[b]["out"] for b in range(B)], axis=0)
    return out, res


def kernel(x, wq, wk, wv, bq, bk, bv, gamma):
    out, _ = run_spmd(x, wq, wk, wv, bq, bk, bv, gamma)
    return out


# revision 17
# speedup vs baseline: 1.7973x; 1.0975x over previous
"""Trainium2 Bass kernel for conv-qkv rank-1 attention.

out = gamma * (q+bq) * sum((k+bk)*(v+bv)) + x, where q,k,v are
per-time-slice 3x3 convs (C=64 -> C=64) of x [B=8, C=64, T=16, W=64, H=64].

Sharding: data-parallel over B across 8 cores (1 example/core), conv
weights replicated. No cross-core communication.

v2 design (vs v1 baseline at ~492us):
- No bias matmul taps: bq/bv folded into the ACT/Pool evacuation bias,
  bk folded into the DVE STT op0-add scalar, gamma folded into wv/bv
  host-side. 18 PE pair-slots per (pair, block) instead of 20.
- Stationary weights in bf16: LDWEIGHTS streams half the bytes (it was
  longer than the 512-row matmul itself), moving x stays f32r.
- Host pads H to 66 so each x slice loads with ONE contiguous
  descriptor per partition (was 64 x 256B strided descriptors -> 57us
  serial startup and 205us of DMA activity).
- W-pad rows zeroed once with on-chip memzero (no zero-vector DMAs).
- Merged [128,512] out-STT (q*s+x for both slices at once), s for the
  hi slice moved 0:63 -> 64:127 with a tiny sbuf-sbuf DMA on the
  vector queue.
- v evacuated by Pool (gpsimd), q by ACT, k*v+reduce and out on DVE.
"""

import os

import numpy as np

import concourse.bacc as bacc
import concourse.bass as bass
import concourse.mybir as mybir
import concourse.tile as tile
from concourse import bass_utils

F32 = mybir.dt.float32
F32R = mybir.dt.float32r
BF16 = mybir.dt.bfloat16
ALU = mybir.AluOpType
ACTF = mybir.ActivationFunctionType

B, C, T, W, H = 8, 64, 16, 64, 64
HP = H + 2                     # host-padded H
WP = W + 2                     # SBUF-padded W rows
NPAIR = T // 2                 # slice pairs per core
RB = 8                         # W-rows per pixel block
NBLK = W // RB                 # pixel blocks per slice
BN = RB * H                    # moving free dim per matmul (512)
NTAP = 9                       # conv taps (no bias tap)

XDT_NAME = os.environ.get("BASS_XDT", "bf16")   # moving/x dtype
# walrus rejects mixed 32/16-bit matmul inputs: stationary follows moving
WDT_NAME = os.environ.get("BASS_WDT", "bf16" if XDT_NAME == "bf16" else "f32r")
# out/qs storage dtype follows x by default
ODT_NAME = os.environ.get("BASS_ODT", XDT_NAME)
# GPSIMD cannot access PSUM (BIR verifier) -> evacuations must use ACT
VEVAC = os.environ.get("BASS_VEVAC", "act")     # pool | act


def _round22(a: np.ndarray) -> np.ndarray:
    """Round fp32 to 11 mantissa bits so the PE's FP22 read-truncation is
    exact (unbiased quantization instead of truncation)."""
    u = np.ascontiguousarray(a, np.float32).view(np.uint32).astype(np.uint64)
    u = ((u + 0x800) & 0xFFFFF000).astype(np.uint32)
    return u.view(np.float32)


def _to_bf16(a: np.ndarray) -> np.ndarray:
    import ml_dtypes
    return np.ascontiguousarray(a, np.float32).astype(ml_dtypes.bfloat16)


def _pack_w(a: np.ndarray) -> np.ndarray:
    return _to_bf16(a) if WDT_NAME == "bf16" else _round22(a)


def _pack_weights(wq, wk, wv, bq, bk, bv, gamma):
    """Pack stationary operands (no bias rows; gamma folded into wv/bv).

    wkq [128, 9, 128]: [Wk | Wq] on both partition halves (k lands on
    psum partitions 0-63 for the DVE accum op, q on 64-127).
    wv2 [128, 9, 64]: gamma*Wv on both halves (M=64).
    bias [128, 3]: col0=bq, col1=bk, col2=gamma*bv, duplicated halves.
    """
    g = float(np.asarray(gamma).reshape(-1)[0])

    def taps(w):  # [O, I, 1, 3, 3] -> [I, 9, O]
        return np.ascontiguousarray(
            w.reshape(C, C, 9).transpose(1, 2, 0), np.float32)

    wq_t, wk_t, wv_t = taps(wq), taps(wk), taps(wv) * g
    wkq = np.zeros((128, NTAP, 128), np.float32)
    wkq[0:64, :, 0:64] = wk_t
    wkq[0:64, :, 64:128] = wq_t
    wkq[64:128, :, 0:64] = wk_t
    wkq[64:128, :, 64:128] = wq_t

    wv2 = np.zeros((128, NTAP, 64), np.float32)
    wv2[0:64] = wv_t
    wv2[64:128] = wv_t

    bias = np.zeros((128, 3), np.float32)
    bias[0:64, 0] = bq
    bias[64:128, 0] = bq
    bias[0:64, 1] = bk
    bias[64:128, 1] = bk
    bias[0:64, 2] = bv * g
    bias[64:128, 2] = bv * g
    return _pack_w(wkq), _pack_w(wv2), bias


def _emit(nc, tc, x_d, wkq_d, wv_d, bias_d, zer_d, out_d, ctx):
    xdt = F32R if XDT_NAME == "f32r" else BF16  # storage dtype of x tiles

    const = ctx.enter_context(tc.tile_pool(name="const", bufs=1))
    state = ctx.enter_context(tc.tile_pool(name="state", bufs=1))
    psum = ctx.enter_context(
        tc.tile_pool(name="psum", bufs=2, space=bass.MemorySpace.PSUM))
    vpool = ctx.enter_context(tc.tile_pool(name="vpool", bufs=2))

    wdt = BF16 if WDT_NAME == "bf16" else F32R
    wkq_t = const.tile([128, NTAP, 128], wdt, tag="wkq")
    wv_t = const.tile([128, NTAP, 64], wdt, tag="wv")
    bias_t = const.tile([128, 3], F32, tag="bias")

    odt = F32 if ODT_NAME == "f32r" else BF16

    xp = [state.tile([128, WP, HP], xdt, tag=f"xp{i}", name=f"xp{i}")
          for i in range(3)]
    qs = [state.tile([128, W * H], odt, tag=f"qs{i}", name=f"qs{i}")
          for i in range(2)]
    ot = [state.tile([128, W * H], odt, tag=f"ot{i}", name=f"ot{i}")
          for i in range(2)]
    scr = [state.tile([64, BN], F32, tag=f"scr{i}", name=f"scr{i}")
           for i in range(2)]
    sparts = [state.tile([64, 2, NBLK], F32, tag=f"sp{i}", name=f"sp{i}")
              for i in range(2)]
    sfull = [state.tile([128, 1], F32, tag=f"sf{i}", name=f"sf{i}")
             for i in range(2)]
    stmp = [state.tile([64, 1], F32, tag=f"st{i}", name=f"st{i}")
            for i in range(2)]

    def load_pair(p):
        t_ = xp[p % 3]
        nc.sync.dma_start(t_[0:64, 1:1 + W, :], x_d[:, 2 * p])
        nc.sync.dma_start(t_[64:128, 1:1 + W, :], x_d[:, 2 * p + 1])

    # weights first on the gpsimd queue (small; gate the first matmul)
    nc.gpsimd.dma_start(wv_t[:], wv_d[:])
    nc.gpsimd.dma_start(wkq_t[:], wkq_d[:])
    nc.gpsimd.dma_start(bias_t[:], bias_d[:])
    # zero the W-pad rows once (H-pad columns come zeroed from the host).
    # The BIR verifier rejects compute-engine writes feeding an fp32r
    # matmul, so in f32r mode the zeros come from a host tensor via DMA
    # (on the ACT queue, off the x-load path).
    for t_ in xp:
        if xdt == F32R:
            nc.scalar.dma_start(t_[:, 0, :], zer_d[:, :])
            nc.scalar.dma_start(t_[:, WP - 1, :], zer_d[:, :])
        else:
            nc.scalar.memzero(t_[:, 0, :])
            nc.scalar.memzero(t_[:, WP - 1, :])
    # first pair split across all three DMA-capable queues so the first
    # matmul can start after a ~2us quarter-slice load
    HW2 = W // 2
    nc.sync.dma_start(xp[0][0:64, 1:1 + HW2, :], x_d[:, 0, 0:HW2])
    nc.scalar.dma_start(xp[0][0:64, 1 + HW2:1 + W, :], x_d[:, 0, HW2:W])
    nc.gpsimd.dma_start(xp[0][64:128, 1:1 + HW2, :], x_d[:, 1, 0:HW2])
    nc.sync.dma_start(xp[0][64:128, 1 + HW2:1 + W, :], x_d[:, 1, HW2:W])
    load_pair(1)

    def mm_rhs(xp_, half, tap, j):
        dy, dx = tap // 3, tap % 3
        r0 = j * RB + dy
        return xp_[64 * half:64 * half + 64, r0:r0 + RB, dx:dx + H]

    for p in range(NPAIR):
        pb = p % 2
        xp_, qs_, ot_ = xp[p % 3], qs[pb], ot[pb]

        if p + 2 < NPAIR:
            load_pair(p + 2)

        for j in range(NBLK):
            v_lo = psum.tile([128, BN], F32, tag="v_lo", name="v_lo")
            v_hi = psum.tile([128, BN], F32, tag="v_hi", name="v_hi")
            kq_lo = psum.tile([128, BN], F32, tag="kq_lo")
            kq_hi = psum.tile([128, BN], F32, tag="kq_hi")

            for tap in range(NTAP):
                st, sp = tap == 0, tap == NTAP - 1
                nc.tensor.matmul(
                    v_lo[0:64, :], wv_t[0:64, tap, :],
                    mm_rhs(xp_, 0, tap, j), start=st, stop=sp)
                nc.tensor.matmul(
                    v_hi[0:64, :], wv_t[64:128, tap, :],
                    mm_rhs(xp_, 1, tap, j), start=st, stop=sp)
            for tap in range(NTAP):
                st, sp = tap == 0, tap == NTAP - 1
                nc.tensor.matmul(
                    kq_lo[:, :], wkq_t[0:64, tap, :],
                    mm_rhs(xp_, 0, tap, j), start=st, stop=sp)
                nc.tensor.matmul(
                    kq_hi[:, :], wkq_t[64:128, tap, :],
                    mm_rhs(xp_, 1, tap, j), start=st, stop=sp)

            # v + bv -> SBUF (Pool), q + bq -> SBUF (ACT; lo crosses
            # partitions 64-127 -> 0-63 to line up with x_t)
            vsb_lo = vpool.tile([64, BN], F32, tag="vsb_lo", name="vsb_lo")
            vsb_hi = vpool.tile([64, BN], F32, tag="vsb_hi", name="vsb_hi")
            if VEVAC == "pool":
                nc.gpsimd.tensor_scalar_add(
                    vsb_lo[:, :], v_lo[0:64, :], bias_t[0:64, 2:3])
                nc.gpsimd.tensor_scalar_add(
                    vsb_hi[:, :], v_hi[0:64, :], bias_t[0:64, 2:3])
            else:
                nc.scalar.activation(
                    vsb_lo[:, :], v_lo[0:64, :], ACTF.Identity,
                    bias=bias_t[0:64, 2:3])
                nc.scalar.activation(
                    vsb_hi[:, :], v_hi[0:64, :], ACTF.Identity,
                    bias=bias_t[0:64, 2:3])
            nc.scalar.activation(
                qs_[0:64, j * BN:(j + 1) * BN], kq_lo[64:128, :],
                ACTF.Identity, bias=bias_t[64:128, 0:1])
            nc.scalar.activation(
                qs_[64:128, j * BN:(j + 1) * BN], kq_hi[64:128, :],
                ACTF.Identity, bias=bias_t[64:128, 0:1])

            # (k+bk)*v with pixel-sum accumulation (k from PSUM base 0)
            nc.vector.scalar_tensor_tensor(
                out=scr[0][:, :], in0=kq_lo[0:64, :],
                scalar=bias_t[0:64, 1:2], in1=vsb_lo[:, :],
                op0=ALU.add, op1=ALU.mult,
                accum_out=sparts[pb][:, 0, j:j + 1])
            nc.vector.scalar_tensor_tensor(
                out=scr[1][:, :], in0=kq_hi[0:64, :],
                scalar=bias_t[0:64, 1:2], in1=vsb_hi[:, :],
                op0=ALU.add, op1=ALU.mult,
                accum_out=sparts[pb][:, 1, j:j + 1])

        nc.vector.tensor_reduce(
            sfull[pb][0:64, 0:1], sparts[pb][:, 0, :],
            axis=mybir.AxisListType.X, op=ALU.add)
        nc.vector.tensor_reduce(
            stmp[pb][:, 0:1], sparts[pb][:, 1, :],
            axis=mybir.AxisListType.X, op=ALU.add)
        # s_{t+1} accumulates on partitions 0-63 but q_{t+1}/x_{t+1} live
        # on 64-127: move it with a tiny sbuf->sbuf DMA on the ACT queue
        nc.scalar.dma_start(sfull[pb][64:128, :], stmp[pb][:, :])

        for j in range(NBLK):
            in1 = xp_[:, 1 + j * RB:1 + (j + 1) * RB, 1:1 + H]
            if xdt == F32R:
                in1 = in1.bitcast(F32)
            nc.vector.scalar_tensor_tensor(
                out=ot_[:, j * BN:(j + 1) * BN],
                in0=qs_[:, j * BN:(j + 1) * BN],
                scalar=sfull[pb][:, 0:1],
                in1=in1,
                op0=ALU.mult, op1=ALU.add)

        # split out-DMAs across queues (sync's x loads are done by the
        # time the last pairs drain, so the tail stays short)
        nc.gpsimd.dma_start(out_d[:, 2 * p], ot_[0:64, :])
        nc.sync.dma_start(out_d[:, 2 * p + 1], ot_[64:128, :])


_CACHE = {}


def _build():
    key = (XDT_NAME, WDT_NAME, VEVAC)
    if key in _CACHE:
        return _CACHE[key]
    nc = bacc.Bacc("TRN2", target_bir_lowering=False, debug=False,
                   enable_asserts=False, num_devices=8)
    xdt = F32R if XDT_NAME == "f32r" else BF16
    wdt = BF16 if WDT_NAME == "bf16" else F32R
    x_d = nc.dram_tensor("xpad", (C, T, W, HP), xdt,
                         kind="ExternalInput").ap()
    wkq_d = nc.dram_tensor("wkq", (128, NTAP, 128), wdt,
                           kind="ExternalInput").ap()
    wv_d = nc.dram_tensor("wv2", (128, NTAP, 64), wdt,
                          kind="ExternalInput").ap()
    bias_d = nc.dram_tensor("biases", (128, 3), F32,
                            kind="ExternalInput").ap()
    zer_d = nc.dram_tensor("zer", (128, HP), xdt,
                           kind="ExternalInput").ap()
    odt = F32 if ODT_NAME == "f32r" else BF16
    out_d = nc.dram_tensor("out", (C, T, W, H), odt,
                           kind="ExternalOutput").ap()
    from contextlib import ExitStack
    with tile.TileContext(nc) as tc, ExitStack() as ctx:
        _emit(nc, tc, x_d, wkq_d, wv_d, bias_d, zer_d, out_d, ctx)
    nc.compile()
    _CACHE[key] = nc
    return nc


def run_spmd(x, wq, wk, wv, bq, bk, bv, gamma, trace=False, **kw):
    nc = _build()
    wkq, wv2, biases = _pack_weights(
        np.asarray(wq, np.float32), np.asarray(wk, np.float32),
        np.asarray(wv, np.float32), np.asarray(bq, np.float32),
        np.asarray(bk, np.float32), np.asarray(bv, np.float32),
        np.asarray(gamma, np.float32))
    x = np.asarray(x, np.float32)
    xpad = np.zeros((B, C, T, W, HP), np.float32)
    xpad[..., 1:1 + H] = x
    zer = np.zeros((128, HP), np.float32)
    if XDT_NAME == "bf16":
        xpad = _to_bf16(xpad)
        zer = _to_bf16(zer)
    in_maps = [
        {"xpad": np.ascontiguousarray(xpad[b]), "wkq": wkq, "wv2": wv2,
         "biases": biases, "zer": zer}
        for b in range(B)
    ]
    res = bass_utils.run_bass_kernel_spmd(
        nc, in_maps, core_ids=list(range(B)), trace=trace, **kw)
    out = np.stack(
        [np.asarray(res.results[b]["out"], np.float32) for b in range(B)],
        axis=0)
    return out, res


def kernel(x, wq, wk, wv, bq, bk, bv, gamma):
    out, _ = run_spmd(x, wq, wk, wv, bq, bk, bv, gamma)
    return out


# revision 25
# speedup vs baseline: 1.8583x; 1.0339x over previous
"""Trainium2 Bass kernel for conv-qkv rank-1 attention.

out = gamma * (q+bq) * sum((k+bk)*(v+bv)) + x, where q,k,v are
per-time-slice 3x3 convs (C=64 -> C=64) of x [B=8, C=64, T=16, W=64, H=64].

Sharding: data-parallel over B across 8 cores (1 example/core), conv
weights replicated. No cross-core communication.

v2 design (vs v1 baseline at ~492us):
- No bias matmul taps: bq/bv folded into the ACT/Pool evacuation bias,
  bk folded into the DVE STT op0-add scalar, gamma folded into wv/bv
  host-side. 18 PE pair-slots per (pair, block) instead of 20.
- Stationary weights in bf16: LDWEIGHTS streams half the bytes (it was
  longer than the 512-row matmul itself), moving x stays f32r.
- Host pads H to 66 so each x slice loads with ONE contiguous
  descriptor per partition (was 64 x 256B strided descriptors -> 57us
  serial startup and 205us of DMA activity).
- W-pad rows zeroed once with on-chip memzero (no zero-vector DMAs).
- Merged [128,512] out-STT (q*s+x for both slices at once), s for the
  hi slice moved 0:63 -> 64:127 with a tiny sbuf-sbuf DMA on the
  vector queue.
- v evacuated by Pool (gpsimd), q by ACT, k*v+reduce and out on DVE.
"""

import os

import numpy as np

import concourse.bacc as bacc
import concourse.bass as bass
import concourse.mybir as mybir
import concourse.tile as tile
from concourse import bass_utils

F32 = mybir.dt.float32
F32R = mybir.dt.float32r
BF16 = mybir.dt.bfloat16
ALU = mybir.AluOpType
ACTF = mybir.ActivationFunctionType

B, C, T, W, H = 8, 64, 16, 64, 64
HP = H + 2                     # host-padded H
WP = W + 2                     # SBUF-padded W rows
NPAIR = T // 2                 # slice pairs per core
RB = 8                         # W-rows per pixel block
NBLK = W // RB                 # pixel blocks per slice
BN = RB * H                    # moving free dim per matmul (512)
NTAP = 9                       # conv taps (no bias tap)

XDT_NAME = os.environ.get("BASS_XDT", "bf16")   # moving/x dtype
# walrus rejects mixed 32/16-bit matmul inputs: stationary follows moving
WDT_NAME = os.environ.get("BASS_WDT", "bf16" if XDT_NAME == "bf16" else "f32r")
# out/qs storage dtype follows x by default
ODT_NAME = os.environ.get("BASS_ODT", XDT_NAME)
# GPSIMD cannot access PSUM (BIR verifier) -> evacuations must use ACT
VEVAC = os.environ.get("BASS_VEVAC", "act")     # pool | act
# v-hi matmuls write psum partitions 64-127 of the same bank as v-lo
# (bf16 permits tile_position col 64), giving one merged v evacuation
VQUAD = os.environ.get("BASS_VQUAD", "1" if XDT_NAME == "bf16" else "0") == "1"
# Pool rejects TensorScalarPtr at codegen -> out-STT stays on DVE
POOLOUT = os.environ.get("BASS_POOLOUT", "0") == "1"


def _round22(a: np.ndarray) -> np.ndarray:
    """Round fp32 to 11 mantissa bits so the PE's FP22 read-truncation is
    exact (unbiased quantization instead of truncation)."""
    u = np.ascontiguousarray(a, np.float32).view(np.uint32).astype(np.uint64)
    u = ((u + 0x800) & 0xFFFFF000).astype(np.uint32)
    return u.view(np.float32)


def _to_bf16(a: np.ndarray) -> np.ndarray:
    import ml_dtypes
    return np.ascontiguousarray(a, np.float32).astype(ml_dtypes.bfloat16)


def _pack_w(a: np.ndarray) -> np.ndarray:
    return _to_bf16(a) if WDT_NAME == "bf16" else _round22(a)


def _pack_weights(wq, wk, wv, bq, bk, bv, gamma):
    """Pack stationary operands (no bias rows; gamma folded into wv/bv).

    wkq [128, 9, 128]: [Wk | Wq] on both partition halves (k lands on
    psum partitions 0-63 for the DVE accum op, q on 64-127).
    wv2 [128, 9, 64]: gamma*Wv on both halves (M=64).
    bias [128, 3]: col0=bq, col1=bk, col2=gamma*bv, duplicated halves.
    """
    g = float(np.asarray(gamma).reshape(-1)[0])

    def taps(w):  # [O, I, 1, 3, 3] -> [I, 9, O]
        return np.ascontiguousarray(
            w.reshape(C, C, 9).transpose(1, 2, 0), np.float32)

    wq_t, wk_t, wv_t = taps(wq), taps(wk), taps(wv) * g
    # lo chain: [Wk | Wq] (k on psum partitions 0-63); hi chain flipped
    # to [Wq | Wk] so k_{t+1} lands on partitions 64-127 and the whole
    # hi k*v/s pipeline stays on the upper partition half (no s swap)
    wkq = np.zeros((128, NTAP, 128), np.float32)
    wkq[0:64, :, 0:64] = wk_t
    wkq[0:64, :, 64:128] = wq_t
    wkq[64:128, :, 0:64] = wq_t
    wkq[64:128, :, 64:128] = wk_t

    wv2 = np.zeros((128, NTAP, 64), np.float32)
    wv2[0:64] = wv_t
    wv2[64:128] = wv_t

    bias = np.zeros((128, 3), np.float32)
    bias[0:64, 0] = bq
    bias[64:128, 0] = bq
    bias[0:64, 1] = bk
    bias[64:128, 1] = bk
    bias[0:64, 2] = bv * g
    bias[64:128, 2] = bv * g
    return _pack_w(wkq), _pack_w(wv2), bias


def _emit(nc, tc, x_d, wkq_d, wv_d, bias_d, zer_d, out_d, ctx):
    xdt = F32R if XDT_NAME == "f32r" else BF16  # storage dtype of x tiles

    const = ctx.enter_context(tc.tile_pool(name="const", bufs=1))
    state = ctx.enter_context(tc.tile_pool(name="state", bufs=1))
    psum = ctx.enter_context(
        tc.tile_pool(name="psum", bufs=2, space=bass.MemorySpace.PSUM))
    vpool = ctx.enter_context(tc.tile_pool(name="vpool", bufs=2))

    wdt = BF16 if WDT_NAME == "bf16" else F32R
    wkq_t = const.tile([128, NTAP, 128], wdt, tag="wkq")
    wv_t = const.tile([128, NTAP, 64], wdt, tag="wv")
    bias_t = const.tile([128, 3], F32, tag="bias")

    odt = F32 if ODT_NAME == "f32r" else BF16

    xp = [state.tile([128, WP, HP], xdt, tag=f"xp{i}", name=f"xp{i}")
          for i in range(3)]
    qs = [state.tile([128, W * H], odt, tag=f"qs{i}", name=f"qs{i}")
          for i in range(2)]
    ot = [state.tile([128, W * H], odt, tag=f"ot{i}", name=f"ot{i}")
          for i in range(2)]
    scr = state.tile([128, BN], F32, tag="scr", name="scr")
    sparts = [state.tile([128, NBLK], F32, tag=f"sp{i}", name=f"sp{i}")
              for i in range(2)]
    sfull = [state.tile([128, 1], F32, tag=f"sf{i}", name=f"sf{i}")
             for i in range(2)]

    def load_pair(p):
        t_ = xp[p % 3]
        nc.sync.dma_start(t_[0:64, 1:1 + W, :], x_d[:, 2 * p])
        nc.sync.dma_start(t_[64:128, 1:1 + W, :], x_d[:, 2 * p + 1])

    # weights first on the gpsimd queue (small; gate the first matmul)
    nc.gpsimd.dma_start(wv_t[:], wv_d[:])
    nc.gpsimd.dma_start(wkq_t[:], wkq_d[:])
    # zero the W-pad rows once (H-pad columns come zeroed from the host).
    # The BIR verifier rejects compute-engine writes feeding an fp32r
    # matmul, so in f32r mode the zeros come from a host tensor via DMA
    # (on the ACT queue, off the x-load path).
    for t_ in xp:
        if xdt == F32R:
            nc.scalar.dma_start(t_[:, 0, :], zer_d[:, :])
            nc.scalar.dma_start(t_[:, WP - 1, :], zer_d[:, :])
        else:
            nc.vector.memset(t_[:, 0, :], 0.0)
            nc.vector.memset(t_[:, WP - 1, :], 0.0)
    # first pair split across all three DMA-capable queues so the first
    # matmul can start after a ~2us quarter-slice load
    HW2 = W // 2
    nc.sync.dma_start(xp[0][0:64, 1:1 + HW2, :], x_d[:, 0, 0:HW2])
    nc.scalar.dma_start(xp[0][0:64, 1 + HW2:1 + W, :], x_d[:, 0, HW2:W])
    nc.gpsimd.dma_start(xp[0][64:128, 1:1 + HW2, :], x_d[:, 1, 0:HW2])
    nc.sync.dma_start(xp[0][64:128, 1 + HW2:1 + W, :], x_d[:, 1, HW2:W])
    nc.gpsimd.dma_start(bias_t[:], bias_d[:])
    load_pair(1)

    def mm_rhs(xp_, half, tap, j):
        dy, dx = tap // 3, tap % 3
        r0 = j * RB + dy
        return xp_[64 * half:64 * half + 64, r0:r0 + RB, dx:dx + H]

    for p in range(NPAIR):
        pb = p % 2
        xp_, qs_, ot_ = xp[p % 3], qs[pb], ot[pb]

        if p + 2 < NPAIR:
            load_pair(p + 2)

        for j in range(NBLK):
            if VQUAD:
                v_lo = v_hi = psum.tile([128, BN], F32, tag="v_lo",
                                        name="v_lo")
                v_hi_out = v_hi[64:128, :]
            else:
                v_lo = psum.tile([128, BN], F32, tag="v_lo", name="v_lo")
                v_hi = psum.tile([128, BN], F32, tag="v_hi", name="v_hi")
                v_hi_out = v_hi[0:64, :]
            kq_lo = psum.tile([128, BN], F32, tag="kq_lo")
            kq_hi = psum.tile([128, BN], F32, tag="kq_hi")

            for tap in range(NTAP):
                st, sp = tap == 0, tap == NTAP - 1
                nc.tensor.matmul(
                    v_lo[0:64, :], wv_t[0:64, tap, :],
                    mm_rhs(xp_, 0, tap, j), start=st, stop=sp)
                nc.tensor.matmul(
                    v_hi_out, wv_t[64:128, tap, :],
                    mm_rhs(xp_, 1, tap, j), start=st, stop=sp)
            for tap in range(NTAP):
                st, sp = tap == 0, tap == NTAP - 1
                nc.tensor.matmul(
                    kq_lo[:, :], wkq_t[0:64, tap, :],
                    mm_rhs(xp_, 0, tap, j), start=st, stop=sp)
                nc.tensor.matmul(
                    kq_hi[:, :], wkq_t[64:128, tap, :],
                    mm_rhs(xp_, 1, tap, j), start=st, stop=sp)

            # v + bv -> SBUF (ACT), q + bq -> SBUF (ACT; lo crosses
            # partitions 64-127 -> 0-63 to line up with x_t)
            vsb = vpool.tile([128, BN], F32, tag="vsb", name="vsb")
            if VQUAD:
                nc.scalar.activation(
                    vsb[:, :], v_lo[:, :], ACTF.Identity,
                    bias=bias_t[:, 2:3])
            else:
                nc.scalar.activation(
                    vsb[0:64, :], v_lo[0:64, :], ACTF.Identity,
                    bias=bias_t[0:64, 2:3])
                nc.scalar.activation(
                    vsb[64:128, :], v_hi[0:64, :], ACTF.Identity,
                    bias=bias_t[64:128, 2:3])
            nc.scalar.activation(
                qs_[0:64, j * BN:(j + 1) * BN], kq_lo[64:128, :],
                ACTF.Identity, bias=bias_t[64:128, 0:1])
            nc.scalar.activation(
                qs_[64:128, j * BN:(j + 1) * BN], kq_hi[0:64, :],
                ACTF.Identity, bias=bias_t[0:64, 0:1])

            # (k+bk)*v with pixel-sum accumulation; the lo chain lives on
            # partitions 0-63, the hi chain on 64-127 throughout
            nc.vector.scalar_tensor_tensor(
                out=scr[0:64, :], in0=kq_lo[0:64, :],
                scalar=bias_t[0:64, 1:2], in1=vsb[0:64, :],
                op0=ALU.add, op1=ALU.mult,
                accum_out=sparts[pb][0:64, j:j + 1])
            nc.vector.scalar_tensor_tensor(
                out=scr[64:128, :], in0=kq_hi[64:128, :],
                scalar=bias_t[64:128, 1:2], in1=vsb[64:128, :],
                op0=ALU.add, op1=ALU.mult,
                accum_out=sparts[pb][64:128, j:j + 1])

        nc.vector.tensor_reduce(
            sfull[pb][:, 0:1], sparts[pb][:, :],
            axis=mybir.AxisListType.X, op=ALU.add)

        for j in range(NBLK):
            in1 = xp_[:, 1 + j * RB:1 + (j + 1) * RB, 1:1 + H]
            if xdt == F32R:
                in1 = in1.bitcast(F32)
            # split the q*s+x epilogue between DVE and the idle Pool
            # engine (all operands in SBUF, so Pool is allowed)
            eng = nc.gpsimd if (POOLOUT and j % 2 == 1) else nc.vector
            eng.scalar_tensor_tensor(
                out=ot_[:, j * BN:(j + 1) * BN],
                in0=qs_[:, j * BN:(j + 1) * BN],
                scalar=sfull[pb][:, 0:1],
                in1=in1,
                op0=ALU.mult, op1=ALU.add)

        # split out-DMAs across queues (sync's x loads are done by the
        # time the last pairs drain, so the tail stays short)
        nc.gpsimd.dma_start(out_d[:, 2 * p], ot_[0:64, :])
        nc.sync.dma_start(out_d[:, 2 * p + 1], ot_[64:128, :])


_CACHE = {}


def _build():
    key = (XDT_NAME, WDT_NAME, VEVAC)
    if key in _CACHE:
        return _CACHE[key]
    nc = bacc.Bacc("TRN2", target_bir_lowering=False, debug=False,
                   enable_asserts=False, num_devices=8)
    xdt = F32R if XDT_NAME == "f32r" else BF16
    wdt = BF16 if WDT_NAME == "bf16" else F32R
    x_d = nc.dram_tensor("xpad", (C, T, W, HP), xdt,
                         kind="ExternalInput").ap()
    wkq_d = nc.dram_tensor("wkq", (128, NTAP, 128), wdt,
                           kind="ExternalInput").ap()
    wv_d = nc.dram_tensor("wv2", (128, NTAP, 64), wdt,
                          kind="ExternalInput").ap()
    bias_d = nc.dram_tensor("biases", (128, 3), F32,
                            kind="ExternalInput").ap()
    zer_d = nc.dram_tensor("zer", (128, HP), xdt,
                           kind="ExternalInput").ap()
    odt = F32 if ODT_NAME == "f32r" else BF16
    out_d = nc.dram_tensor("out", (C, T, W, H), odt,
                           kind="ExternalOutput").ap()
    from contextlib import ExitStack
    with tile.TileContext(nc) as tc, ExitStack() as ctx:
        _emit(nc, tc, x_d, wkq_d, wv_d, bias_d, zer_d, out_d, ctx)
    nc.compile()
    _CACHE[key] = nc
    return nc


def run_spmd(x, wq, wk, wv, bq, bk, bv, gamma, trace=False, **kw):
    nc = _build()
    wkq, wv2, biases = _pack_weights(
        np.asarray(wq, np.float32), np.asarray(wk, np.float32),
        np.asarray(wv, np.float32), np.asarray(bq, np.float32),
        np.asarray(bk, np.float32), np.asarray(bv, np.float32),
        np.asarray(gamma, np.float32))
    x = np.asarray(x, np.float32)
    xpad = np.zeros((B, C, T, W, HP), np.float32)
    xpad[..., 1:1 + H] = x
    zer = np.zeros((128, HP), np.float32)
    if XDT_NAME == "bf16":
        xpad = _to_bf16(xpad)
        zer = _to_bf16(zer)
    in_maps = [
        {"xpad": np.ascontiguousarray(xpad[b]), "wkq": wkq, "wv2": wv2,
         "biases": biases, "zer": zer}
        for b in range(B)
    ]
    res = bass_utils.run_bass_kernel_spmd(
        nc, in_maps, core_ids=list(range(B)), trace=trace, **kw)
    out = np.stack(
        [np.asarray(res.results[b]["out"], np.float32) for b in range(B)],
        axis=0)
    return out, res


def kernel(x, wq, wk, wv, bq, bk, bv, gamma):
    out, _ = run_spmd(x, wq, wk, wv, bq, bk, bv, gamma)
    return out


# revision 29
# speedup vs baseline: 1.8903x; 1.0172x over previous
"""Trainium2 Bass kernel for conv-qkv rank-1 attention.

out = gamma * (q+bq) * sum((k+bk)*(v+bv)) + x, where q,k,v are
per-time-slice 3x3 convs (C=64 -> C=64) of x [B=8, C=64, T=16, W=64, H=64].

Sharding: data-parallel over B across 8 cores (1 example/core), conv
weights replicated. No cross-core communication.

v2 design (vs v1 baseline at ~492us):
- No bias matmul taps: bq/bv folded into the ACT/Pool evacuation bias,
  bk folded into the DVE STT op0-add scalar, gamma folded into wv/bv
  host-side. 18 PE pair-slots per (pair, block) instead of 20.
- Stationary weights in bf16: LDWEIGHTS streams half the bytes (it was
  longer than the 512-row matmul itself), moving x stays f32r.
- Host pads H to 66 so each x slice loads with ONE contiguous
  descriptor per partition (was 64 x 256B strided descriptors -> 57us
  serial startup and 205us of DMA activity).
- W-pad rows zeroed once with on-chip memzero (no zero-vector DMAs).
- Merged [128,512] out-STT (q*s+x for both slices at once), s for the
  hi slice moved 0:63 -> 64:127 with a tiny sbuf-sbuf DMA on the
  vector queue.
- v evacuated by Pool (gpsimd), q by ACT, k*v+reduce and out on DVE.
"""

import os

import numpy as np

import concourse.bacc as bacc
import concourse.bass as bass
import concourse.mybir as mybir
import concourse.tile as tile
from concourse import bass_utils

F32 = mybir.dt.float32
F32R = mybir.dt.float32r
BF16 = mybir.dt.bfloat16
ALU = mybir.AluOpType
ACTF = mybir.ActivationFunctionType

B, C, T, W, H = 8, 64, 16, 64, 64
HP = H + 2                     # host-padded H
WP = W + 2                     # SBUF-padded W rows
NPAIR = T // 2                 # slice pairs per core
RB = 8                         # W-rows per pixel block
NBLK = W // RB                 # pixel blocks per slice
BN = RB * H                    # moving free dim per matmul (512)
NTAP = 9                       # conv taps (no bias tap)

XDT_NAME = os.environ.get("BASS_XDT", "bf16")   # moving/x dtype
# walrus rejects mixed 32/16-bit matmul inputs: stationary follows moving
WDT_NAME = os.environ.get("BASS_WDT", "bf16" if XDT_NAME == "bf16" else "f32r")
# out/qs storage dtype follows x by default
ODT_NAME = os.environ.get("BASS_ODT", XDT_NAME)
# GPSIMD cannot access PSUM (BIR verifier) -> evacuations must use ACT
VEVAC = os.environ.get("BASS_VEVAC", "act")     # pool | act
# v-hi matmuls write psum partitions 64-127 of the same bank as v-lo
# (bf16 permits tile_position col 64), giving one merged v evacuation
VQUAD = os.environ.get("BASS_VQUAD", "1" if XDT_NAME == "bf16" else "0") == "1"
# Pool rejects TensorScalarPtr at codegen -> out-STT stays on DVE
POOLOUT = os.environ.get("BASS_POOLOUT", "0") == "1"


def _round22(a: np.ndarray) -> np.ndarray:
    """Round fp32 to 11 mantissa bits so the PE's FP22 read-truncation is
    exact (unbiased quantization instead of truncation)."""
    u = np.ascontiguousarray(a, np.float32).view(np.uint32).astype(np.uint64)
    u = ((u + 0x800) & 0xFFFFF000).astype(np.uint32)
    return u.view(np.float32)


def _to_bf16(a: np.ndarray) -> np.ndarray:
    import ml_dtypes
    return np.ascontiguousarray(a, np.float32).astype(ml_dtypes.bfloat16)


def _pack_w(a: np.ndarray) -> np.ndarray:
    return _to_bf16(a) if WDT_NAME == "bf16" else _round22(a)


def _pack_weights(wq, wk, wv, bq, bk, bv, gamma):
    """Pack stationary operands (no bias rows; gamma folded into wv/bv).

    wkq [128, 9, 128]: [Wk | Wq] on both partition halves (k lands on
    psum partitions 0-63 for the DVE accum op, q on 64-127).
    wv2 [128, 9, 64]: gamma*Wv on both halves (M=64).
    bias [128, 3]: col0=bq, col1=bk, col2=gamma*bv, duplicated halves.
    """
    g = float(np.asarray(gamma).reshape(-1)[0])

    def taps(w):  # [O, I, 1, 3, 3] -> [I, 9, O]
        return np.ascontiguousarray(
            w.reshape(C, C, 9).transpose(1, 2, 0), np.float32)

    wq_t, wk_t, wv_t = taps(wq), taps(wk), taps(wv) * g
    # lo chain: [Wk | Wq] (k on psum partitions 0-63); hi chain flipped
    # to [Wq | Wk] so k_{t+1} lands on partitions 64-127 and the whole
    # hi k*v/s pipeline stays on the upper partition half (no s swap)
    wkq = np.zeros((128, NTAP, 128), np.float32)
    wkq[0:64, :, 0:64] = wk_t
    wkq[0:64, :, 64:128] = wq_t
    wkq[64:128, :, 0:64] = wq_t
    wkq[64:128, :, 64:128] = wk_t

    wv2 = np.zeros((128, NTAP, 64), np.float32)
    wv2[0:64] = wv_t
    wv2[64:128] = wv_t

    bias = np.zeros((128, 3), np.float32)
    bias[0:64, 0] = bq
    bias[64:128, 0] = bq
    bias[0:64, 1] = bk
    bias[64:128, 1] = bk
    bias[0:64, 2] = bv * g
    bias[64:128, 2] = bv * g
    return _pack_w(wkq), _pack_w(wv2), bias


def _emit(nc, tc, x_d, wkq_d, wv_d, bias_d, zer_d, out_d, ctx):
    xdt = F32R if XDT_NAME == "f32r" else BF16  # storage dtype of x tiles

    const = ctx.enter_context(tc.tile_pool(name="const", bufs=1))
    state = ctx.enter_context(tc.tile_pool(name="state", bufs=1))
    psum = ctx.enter_context(
        tc.tile_pool(name="psum", bufs=2, space=bass.MemorySpace.PSUM))
    vpool = ctx.enter_context(tc.tile_pool(name="vpool", bufs=2))

    wdt = BF16 if WDT_NAME == "bf16" else F32R
    wkq_t = const.tile([128, NTAP, 128], wdt, tag="wkq")
    wv_t = const.tile([128, NTAP, 64], wdt, tag="wv")
    bias_t = const.tile([128, 3], F32, tag="bias")

    odt = F32 if ODT_NAME == "f32r" else BF16

    xp = [state.tile([128, WP, HP], xdt, tag=f"xp{i}", name=f"xp{i}")
          for i in range(3)]
    qs = [state.tile([128, W * H], odt, tag=f"qs{i}", name=f"qs{i}")
          for i in range(2)]
    ot = [state.tile([128, W * H], odt, tag=f"ot{i}", name=f"ot{i}")
          for i in range(2)]
    scr = state.tile([128, BN], F32, tag="scr", name="scr")
    sparts = [state.tile([128, NBLK], F32, tag=f"sp{i}", name=f"sp{i}")
              for i in range(2)]
    sfull = [state.tile([128, 1], F32, tag=f"sf{i}", name=f"sf{i}")
             for i in range(2)]

    def load_pair(p):
        t_ = xp[p % 3]
        nc.sync.dma_start(t_[0:64, 1:1 + W, :], x_d[:, 2 * p])
        nc.sync.dma_start(t_[64:128, 1:1 + W, :], x_d[:, 2 * p + 1])

    # weights first on the gpsimd queue (small; gate the first matmul)
    nc.gpsimd.dma_start(wv_t[:], wv_d[:])
    # zero the W-pad rows once (H-pad columns come zeroed from the host).
    # The BIR verifier rejects compute-engine writes feeding an fp32r
    # matmul, so in f32r mode the zeros come from a host tensor via DMA
    # (on the ACT queue, off the x-load path).
    for t_ in xp:
        if xdt == F32R:
            nc.scalar.dma_start(t_[:, 0, :], zer_d[:, :])
            nc.scalar.dma_start(t_[:, WP - 1, :], zer_d[:, :])
        else:
            nc.vector.memset(t_[:, 0, :], 0.0)
            nc.vector.memset(t_[:, WP - 1, :], 0.0)
    # first pair split across all three DMA-capable queues so the first
    # matmul can start after a ~2us quarter-slice load
    HW2 = W // 2
    nc.sync.dma_start(xp[0][0:64, 1:1 + HW2, :], x_d[:, 0, 0:HW2])
    nc.scalar.dma_start(xp[0][0:64, 1 + HW2:1 + W, :], x_d[:, 0, HW2:W])
    nc.gpsimd.dma_start(xp[0][64:128, 1:1 + HW2, :], x_d[:, 1, 0:HW2])
    nc.sync.dma_start(xp[0][64:128, 1 + HW2:1 + W, :], x_d[:, 1, HW2:W])
    nc.gpsimd.dma_start(wkq_t[:], wkq_d[:])
    nc.gpsimd.dma_start(bias_t[:], bias_d[:])
    # load_pair(1) is emitted inside pair 0's block loop: matmuls wait on
    # the issuing queue's DMA counter, so any DMA emitted earlier on the
    # same queue delays the first matmul

    def mm_rhs(xp_, half, tap, j):
        dy, dx = tap // 3, tap % 3
        r0 = j * RB + dy
        return xp_[64 * half:64 * half + 64, r0:r0 + RB, dx:dx + H]

    for p in range(NPAIR):
        pb = p % 2
        xp_, qs_, ot_ = xp[p % 3], qs[pb], ot[pb]

        if p + 2 < NPAIR:
            load_pair(p + 2)

        for j in range(NBLK):
            if p == 0 and j == 2:
                load_pair(1)
            if VQUAD:
                v_lo = v_hi = psum.tile([128, BN], F32, tag="v_lo",
                                        name="v_lo")
                v_hi_out = v_hi[64:128, :]
            else:
                v_lo = psum.tile([128, BN], F32, tag="v_lo", name="v_lo")
                v_hi = psum.tile([128, BN], F32, tag="v_hi", name="v_hi")
                v_hi_out = v_hi[0:64, :]
            kq_lo = psum.tile([128, BN], F32, tag="kq_lo")
            kq_hi = psum.tile([128, BN], F32, tag="kq_hi")

            for tap in range(NTAP):
                st, sp = tap == 0, tap == NTAP - 1
                nc.tensor.matmul(
                    v_lo[0:64, :], wv_t[0:64, tap, :],
                    mm_rhs(xp_, 0, tap, j), start=st, stop=sp)
                nc.tensor.matmul(
                    v_hi_out, wv_t[64:128, tap, :],
                    mm_rhs(xp_, 1, tap, j), start=st, stop=sp)
            for tap in range(NTAP):
                st, sp = tap == 0, tap == NTAP - 1
                nc.tensor.matmul(
                    kq_lo[:, :], wkq_t[0:64, tap, :],
                    mm_rhs(xp_, 0, tap, j), start=st, stop=sp)
                nc.tensor.matmul(
                    kq_hi[:, :], wkq_t[64:128, tap, :],
                    mm_rhs(xp_, 1, tap, j), start=st, stop=sp)

            # v + bv -> SBUF (ACT), q + bq -> SBUF (ACT; lo crosses
            # partitions 64-127 -> 0-63 to line up with x_t)
            vsb = vpool.tile([128, BN], F32, tag="vsb", name="vsb")
            if VQUAD:
                nc.scalar.activation(
                    vsb[:, :], v_lo[:, :], ACTF.Identity,
                    bias=bias_t[:, 2:3])
            else:
                nc.scalar.activation(
                    vsb[0:64, :], v_lo[0:64, :], ACTF.Identity,
                    bias=bias_t[0:64, 2:3])
                nc.scalar.activation(
                    vsb[64:128, :], v_hi[0:64, :], ACTF.Identity,
                    bias=bias_t[64:128, 2:3])
            nc.scalar.activation(
                qs_[0:64, j * BN:(j + 1) * BN], kq_lo[64:128, :],
                ACTF.Identity, bias=bias_t[64:128, 0:1])
            nc.scalar.activation(
                qs_[64:128, j * BN:(j + 1) * BN], kq_hi[0:64, :],
                ACTF.Identity, bias=bias_t[0:64, 0:1])

            # (k+bk)*v with pixel-sum accumulation; the lo chain lives on
            # partitions 0-63, the hi chain on 64-127 throughout
            nc.vector.scalar_tensor_tensor(
                out=scr[0:64, :], in0=kq_lo[0:64, :],
                scalar=bias_t[0:64, 1:2], in1=vsb[0:64, :],
                op0=ALU.add, op1=ALU.mult,
                accum_out=sparts[pb][0:64, j:j + 1])
            nc.vector.scalar_tensor_tensor(
                out=scr[64:128, :], in0=kq_hi[64:128, :],
                scalar=bias_t[64:128, 1:2], in1=vsb[64:128, :],
                op0=ALU.add, op1=ALU.mult,
                accum_out=sparts[pb][64:128, j:j + 1])

        nc.vector.tensor_reduce(
            sfull[pb][:, 0:1], sparts[pb][:, :],
            axis=mybir.AxisListType.X, op=ALU.add)

        for j in range(NBLK):
            in1 = xp_[:, 1 + j * RB:1 + (j + 1) * RB, 1:1 + H]
            if xdt == F32R:
                in1 = in1.bitcast(F32)
            # split the q*s+x epilogue between DVE and the idle Pool
            # engine (all operands in SBUF, so Pool is allowed)
            eng = nc.gpsimd if (POOLOUT and j % 2 == 1) else nc.vector
            eng.scalar_tensor_tensor(
                out=ot_[:, j * BN:(j + 1) * BN],
                in0=qs_[:, j * BN:(j + 1) * BN],
                scalar=sfull[pb][:, 0:1],
                in1=in1,
                op0=ALU.mult, op1=ALU.add)

        # out-DMAs on sync (x loads leave it mostly idle; keeping gpsimd
        # DMA-free after startup dodges its slow end-of-kernel drain)
        nc.sync.dma_start(out_d[:, 2 * p], ot_[0:64, :])
        nc.sync.dma_start(out_d[:, 2 * p + 1], ot_[64:128, :])


_CACHE = {}


def _build():
    key = (XDT_NAME, WDT_NAME, VEVAC)
    if key in _CACHE:
        return _CACHE[key]
    nc = bacc.Bacc("TRN2", target_bir_lowering=False, debug=False,
                   enable_asserts=False, num_devices=8)
    xdt = F32R if XDT_NAME == "f32r" else BF16
    wdt = BF16 if WDT_NAME == "bf16" else F32R
    x_d = nc.dram_tensor("xpad", (C, T, W, HP), xdt,
                         kind="ExternalInput").ap()
    wkq_d = nc.dram_tensor("wkq", (128, NTAP, 128), wdt,
                           kind="ExternalInput").ap()
    wv_d = nc.dram_tensor("wv2", (128, NTAP, 64), wdt,
                          kind="ExternalInput").ap()
    bias_d = nc.dram_tensor("biases", (128, 3), F32,
                            kind="ExternalInput").ap()
    zer_d = nc.dram_tensor("zer", (128, HP), xdt,
                           kind="ExternalInput").ap()
    odt = F32 if ODT_NAME == "f32r" else BF16
    out_d = nc.dram_tensor("out", (C, T, W, H), odt,
                           kind="ExternalOutput").ap()
    from contextlib import ExitStack
    with tile.TileContext(nc) as tc, ExitStack() as ctx:
        _emit(nc, tc, x_d, wkq_d, wv_d, bias_d, zer_d, out_d, ctx)
    nc.compile()
    _CACHE[key] = nc
    return nc


def run_spmd(x, wq, wk, wv, bq, bk, bv, gamma, trace=False, **kw):
    nc = _build()
    wkq, wv2, biases = _pack_weights(
        np.asarray(wq, np.float32), np.asarray(wk, np.float32),
        np.asarray(wv, np.float32), np.asarray(bq, np.float32),
        np.asarray(bk, np.float32), np.asarray(bv, np.float32),
        np.asarray(gamma, np.float32))
    x = np.asarray(x, np.float32)
    xpad = np.zeros((B, C, T, W, HP), np.float32)
    xpad[..., 1:1 + H] = x
    zer = np.zeros((128, HP), np.float32)
    if XDT_NAME == "bf16":
        xpad = _to_bf16(xpad)
        zer = _to_bf16(zer)
    in_maps = [
        {"xpad": np.ascontiguousarray(xpad[b]), "wkq": wkq, "wv2": wv2,
         "biases": biases, "zer": zer}
        for b in range(B)
    ]
    res = bass_utils.run_bass_kernel_spmd(
        nc, in_maps, core_ids=list(range(B)), trace=trace, **kw)
    out = np.stack(
        [np.asarray(res.results[b]["out"], np.float32) for b in range(B)],
        axis=0)
    return out, res


def kernel(x, wq, wk, wv, bq, bk, bv, gamma):
    out, _ = run_spmd(x, wq, wk, wv, bq, bk, bv, gamma)
    return out


# revision 32
# speedup vs baseline: 1.8965x; 1.0033x over previous
"""Trainium2 Bass kernel for conv-qkv rank-1 attention.

out = gamma * (q+bq) * sum((k+bk)*(v+bv)) + x, where q,k,v are
per-time-slice 3x3 convs (C=64 -> C=64) of x [B=8, C=64, T=16, W=64, H=64].

Sharding: data-parallel over B across 8 cores (1 example/core), conv
weights replicated. No cross-core communication.

v2 design (vs v1 baseline at ~492us):
- No bias matmul taps: bq/bv folded into the ACT/Pool evacuation bias,
  bk folded into the DVE STT op0-add scalar, gamma folded into wv/bv
  host-side. 18 PE pair-slots per (pair, block) instead of 20.
- Stationary weights in bf16: LDWEIGHTS streams half the bytes (it was
  longer than the 512-row matmul itself), moving x stays f32r.
- Host pads H to 66 so each x slice loads with ONE contiguous
  descriptor per partition (was 64 x 256B strided descriptors -> 57us
  serial startup and 205us of DMA activity).
- W-pad rows zeroed once with on-chip memzero (no zero-vector DMAs).
- Merged [128,512] out-STT (q*s+x for both slices at once), s for the
  hi slice moved 0:63 -> 64:127 with a tiny sbuf-sbuf DMA on the
  vector queue.
- v evacuated by Pool (gpsimd), q by ACT, k*v+reduce and out on DVE.
"""

import os

import numpy as np

import concourse.bacc as bacc
import concourse.bass as bass
import concourse.mybir as mybir
import concourse.tile as tile
from concourse import bass_utils

F32 = mybir.dt.float32
F32R = mybir.dt.float32r
BF16 = mybir.dt.bfloat16
ALU = mybir.AluOpType
ACTF = mybir.ActivationFunctionType

B, C, T, W, H = 8, 64, 16, 64, 64
HP = H + 2                     # host-padded H
WP = W + 2                     # SBUF-padded W rows
NPAIR = T // 2                 # slice pairs per core
RB = 8                         # W-rows per pixel block
NBLK = W // RB                 # pixel blocks per slice
BN = RB * H                    # moving free dim per matmul (512)
NTAP = 9                       # conv taps (no bias tap)

XDT_NAME = os.environ.get("BASS_XDT", "bf16")   # moving/x dtype
# walrus rejects mixed 32/16-bit matmul inputs: stationary follows moving
WDT_NAME = os.environ.get("BASS_WDT", "bf16" if XDT_NAME == "bf16" else "f32r")
# out/qs storage dtype follows x by default
ODT_NAME = os.environ.get("BASS_ODT", XDT_NAME)
# GPSIMD cannot access PSUM (BIR verifier) -> evacuations must use ACT
VEVAC = os.environ.get("BASS_VEVAC", "act")     # pool | act
# v-hi matmuls write psum partitions 64-127 of the same bank as v-lo
# (bf16 permits tile_position col 64), giving one merged v evacuation
VQUAD = os.environ.get("BASS_VQUAD", "1" if XDT_NAME == "bf16" else "0") == "1"
# Pool rejects TensorScalarPtr at codegen -> out-STT stays on DVE
POOLOUT = os.environ.get("BASS_POOLOUT", "0") == "1"


def _round22(a: np.ndarray) -> np.ndarray:
    """Round fp32 to 11 mantissa bits so the PE's FP22 read-truncation is
    exact (unbiased quantization instead of truncation)."""
    u = np.ascontiguousarray(a, np.float32).view(np.uint32).astype(np.uint64)
    u = ((u + 0x800) & 0xFFFFF000).astype(np.uint32)
    return u.view(np.float32)


def _to_bf16(a: np.ndarray) -> np.ndarray:
    import ml_dtypes
    return np.ascontiguousarray(a, np.float32).astype(ml_dtypes.bfloat16)


def _pack_w(a: np.ndarray) -> np.ndarray:
    return _to_bf16(a) if WDT_NAME == "bf16" else _round22(a)


def _pack_weights(wq, wk, wv, bq, bk, bv, gamma):
    """Pack stationary operands (no bias rows; gamma folded into wv/bv).

    wkq [128, 9, 128]: [Wk | Wq] on both partition halves (k lands on
    psum partitions 0-63 for the DVE accum op, q on 64-127).
    wv2 [128, 9, 64]: gamma*Wv on both halves (M=64).
    bias [128, 3]: col0=bq, col1=bk, col2=gamma*bv, duplicated halves.
    """
    g = float(np.asarray(gamma).reshape(-1)[0])

    def taps(w):  # [O, I, 1, 3, 3] -> [I, 9, O]
        return np.ascontiguousarray(
            w.reshape(C, C, 9).transpose(1, 2, 0), np.float32)

    wq_t, wk_t, wv_t = taps(wq), taps(wk), taps(wv) * g
    # lo chain: [Wk | Wq] (k on psum partitions 0-63); hi chain flipped
    # to [Wq | Wk] so k_{t+1} lands on partitions 64-127 and the whole
    # hi k*v/s pipeline stays on the upper partition half (no s swap)
    wkq = np.zeros((128, NTAP, 128), np.float32)
    wkq[0:64, :, 0:64] = wk_t
    wkq[0:64, :, 64:128] = wq_t
    wkq[64:128, :, 0:64] = wq_t
    wkq[64:128, :, 64:128] = wk_t

    wv2 = np.zeros((128, NTAP, 64), np.float32)
    wv2[0:64] = wv_t
    wv2[64:128] = wv_t

    bias = np.zeros((128, 3), np.float32)
    bias[0:64, 0] = bq
    bias[64:128, 0] = bq
    bias[0:64, 1] = bk
    bias[64:128, 1] = bk
    bias[0:64, 2] = bv * g
    bias[64:128, 2] = bv * g
    return _pack_w(wkq), _pack_w(wv2), bias


def _emit(nc, tc, x_d, wkq_d, wv_d, bias_d, zer_d, out_d, ctx):
    xdt = F32R if XDT_NAME == "f32r" else BF16  # storage dtype of x tiles

    const = ctx.enter_context(tc.tile_pool(name="const", bufs=1))
    state = ctx.enter_context(tc.tile_pool(name="state", bufs=1))
    # kq banks triple-buffered (6) + v shared bank double-buffered (2);
    # without VQUAD, v needs 2 banks/iter so kq drops to double-buffered
    psum = ctx.enter_context(
        tc.tile_pool(name="psum", bufs=3 if VQUAD else 2,
                     space=bass.MemorySpace.PSUM))
    psumv = ctx.enter_context(
        tc.tile_pool(name="psumv", bufs=2, space=bass.MemorySpace.PSUM))
    vpool = ctx.enter_context(tc.tile_pool(name="vpool", bufs=2))

    wdt = BF16 if WDT_NAME == "bf16" else F32R
    wkq_t = const.tile([128, NTAP, 128], wdt, tag="wkq")
    wv_t = const.tile([128, NTAP, 64], wdt, tag="wv")
    bias_t = const.tile([128, 3], F32, tag="bias")

    odt = F32 if ODT_NAME == "f32r" else BF16

    xp = [state.tile([128, WP, HP], xdt, tag=f"xp{i}", name=f"xp{i}")
          for i in range(3)]
    qs = [state.tile([128, W * H], odt, tag=f"qs{i}", name=f"qs{i}")
          for i in range(2)]
    ot = [state.tile([128, W * H], odt, tag=f"ot{i}", name=f"ot{i}")
          for i in range(2)]
    scr = state.tile([128, BN], F32, tag="scr", name="scr")
    sparts = [state.tile([128, NBLK], F32, tag=f"sp{i}", name=f"sp{i}")
              for i in range(2)]
    sfull = [state.tile([128, 1], F32, tag=f"sf{i}", name=f"sf{i}")
             for i in range(2)]

    def load_pair(p):
        t_ = xp[p % 3]
        nc.sync.dma_start(t_[0:64, 1:1 + W, :], x_d[:, 2 * p])
        nc.sync.dma_start(t_[64:128, 1:1 + W, :], x_d[:, 2 * p + 1])

    # weights first on the gpsimd queue (small; gate the first matmul)
    nc.gpsimd.dma_start(wv_t[:], wv_d[:])
    # zero the W-pad rows once (H-pad columns come zeroed from the host).
    # The BIR verifier rejects compute-engine writes feeding an fp32r
    # matmul, so in f32r mode the zeros come from a host tensor via DMA
    # (on the ACT queue, off the x-load path).
    for t_ in xp:
        if xdt == F32R:
            nc.scalar.dma_start(t_[:, 0, :], zer_d[:, :])
            nc.scalar.dma_start(t_[:, WP - 1, :], zer_d[:, :])
        else:
            nc.vector.memset(t_[:, 0, :], 0.0)
            nc.vector.memset(t_[:, WP - 1, :], 0.0)
    # first pair split across all three DMA-capable queues so the first
    # matmul can start after a ~2us quarter-slice load
    HW2 = W // 2
    nc.sync.dma_start(xp[0][0:64, 1:1 + HW2, :], x_d[:, 0, 0:HW2])
    nc.scalar.dma_start(xp[0][0:64, 1 + HW2:1 + W, :], x_d[:, 0, HW2:W])
    nc.gpsimd.dma_start(xp[0][64:128, 1:1 + HW2, :], x_d[:, 1, 0:HW2])
    nc.sync.dma_start(xp[0][64:128, 1 + HW2:1 + W, :], x_d[:, 1, HW2:W])
    nc.gpsimd.dma_start(wkq_t[:], wkq_d[:])
    nc.gpsimd.dma_start(bias_t[:], bias_d[:])
    # load_pair(1) is emitted inside pair 0's block loop: matmuls wait on
    # the issuing queue's DMA counter, so any DMA emitted earlier on the
    # same queue delays the first matmul

    def mm_rhs(xp_, half, tap, j):
        dy, dx = tap // 3, tap % 3
        r0 = j * RB + dy
        return xp_[64 * half:64 * half + 64, r0:r0 + RB, dx:dx + H]

    for p in range(NPAIR):
        pb = p % 2
        xp_, qs_, ot_ = xp[p % 3], qs[pb], ot[pb]

        if p + 2 < NPAIR:
            load_pair(p + 2)

        for j in range(NBLK):
            if p == 0 and j == 2:
                load_pair(1)
            if p == 0 and j == 5:
                load_pair(2)
            if VQUAD:
                v_lo = v_hi = psumv.tile([128, BN], F32, tag="v_lo",
                                         name="v_lo")
                v_hi_out = v_hi[64:128, :]
            else:
                v_lo = psumv.tile([128, BN], F32, tag="v_lo", name="v_lo")
                v_hi = psumv.tile([128, BN], F32, tag="v_hi", name="v_hi")
                v_hi_out = v_hi[0:64, :]
            kq_lo = psum.tile([128, BN], F32, tag="kq_lo")
            kq_hi = psum.tile([128, BN], F32, tag="kq_hi")

            for tap in range(NTAP):
                st, sp = tap == 0, tap == NTAP - 1
                nc.tensor.matmul(
                    v_lo[0:64, :], wv_t[0:64, tap, :],
                    mm_rhs(xp_, 0, tap, j), start=st, stop=sp)
                nc.tensor.matmul(
                    v_hi_out, wv_t[64:128, tap, :],
                    mm_rhs(xp_, 1, tap, j), start=st, stop=sp)
            for tap in range(NTAP):
                st, sp = tap == 0, tap == NTAP - 1
                nc.tensor.matmul(
                    kq_lo[:, :], wkq_t[0:64, tap, :],
                    mm_rhs(xp_, 0, tap, j), start=st, stop=sp)
                nc.tensor.matmul(
                    kq_hi[:, :], wkq_t[64:128, tap, :],
                    mm_rhs(xp_, 1, tap, j), start=st, stop=sp)

            # v + bv -> SBUF (ACT), q + bq -> SBUF (ACT; lo crosses
            # partitions 64-127 -> 0-63 to line up with x_t)
            vsb = vpool.tile([128, BN], F32, tag="vsb", name="vsb")
            if VQUAD:
                nc.scalar.activation(
                    vsb[:, :], v_lo[:, :], ACTF.Identity,
                    bias=bias_t[:, 2:3])
            else:
                nc.scalar.activation(
                    vsb[0:64, :], v_lo[0:64, :], ACTF.Identity,
                    bias=bias_t[0:64, 2:3])
                nc.scalar.activation(
                    vsb[64:128, :], v_hi[0:64, :], ACTF.Identity,
                    bias=bias_t[64:128, 2:3])
            nc.scalar.activation(
                qs_[0:64, j * BN:(j + 1) * BN], kq_lo[64:128, :],
                ACTF.Identity, bias=bias_t[64:128, 0:1])
            nc.scalar.activation(
                qs_[64:128, j * BN:(j + 1) * BN], kq_hi[0:64, :],
                ACTF.Identity, bias=bias_t[0:64, 0:1])

            # (k+bk)*v with pixel-sum accumulation; the lo chain lives on
            # partitions 0-63, the hi chain on 64-127 throughout
            nc.vector.scalar_tensor_tensor(
                out=scr[0:64, :], in0=kq_lo[0:64, :],
                scalar=bias_t[0:64, 1:2], in1=vsb[0:64, :],
                op0=ALU.add, op1=ALU.mult,
                accum_out=sparts[pb][0:64, j:j + 1])
            nc.vector.scalar_tensor_tensor(
                out=scr[64:128, :], in0=kq_hi[64:128, :],
                scalar=bias_t[64:128, 1:2], in1=vsb[64:128, :],
                op0=ALU.add, op1=ALU.mult,
                accum_out=sparts[pb][64:128, j:j + 1])

        nc.vector.tensor_reduce(
            sfull[pb][:, 0:1], sparts[pb][:, :],
            axis=mybir.AxisListType.X, op=ALU.add)

        for j in range(NBLK):
            in1 = xp_[:, 1 + j * RB:1 + (j + 1) * RB, 1:1 + H]
            if xdt == F32R:
                in1 = in1.bitcast(F32)
            # split the q*s+x epilogue between DVE and the idle Pool
            # engine (all operands in SBUF, so Pool is allowed)
            eng = nc.gpsimd if (POOLOUT and j % 2 == 1) else nc.vector
            eng.scalar_tensor_tensor(
                out=ot_[:, j * BN:(j + 1) * BN],
                in0=qs_[:, j * BN:(j + 1) * BN],
                scalar=sfull[pb][:, 0:1],
                in1=in1,
                op0=ALU.mult, op1=ALU.add)

        # out-DMAs on sync (x loads leave it mostly idle; keeping gpsimd
        # DMA-free after startup dodges its slow end-of-kernel drain)
        nc.sync.dma_start(out_d[:, 2 * p], ot_[0:64, :])
        nc.sync.dma_start(out_d[:, 2 * p + 1], ot_[64:128, :])


_CACHE = {}


def _build():
    key = (XDT_NAME, WDT_NAME, VEVAC)
    if key in _CACHE:
        return _CACHE[key]
    nc = bacc.Bacc("TRN2", target_bir_lowering=False, debug=False,
                   enable_asserts=False, num_devices=8)
    xdt = F32R if XDT_NAME == "f32r" else BF16
    wdt = BF16 if WDT_NAME == "bf16" else F32R
    x_d = nc.dram_tensor("xpad", (C, T, W, HP), xdt,
                         kind="ExternalInput").ap()
    wkq_d = nc.dram_tensor("wkq", (128, NTAP, 128), wdt,
                           kind="ExternalInput").ap()
    wv_d = nc.dram_tensor("wv2", (128, NTAP, 64), wdt,
                          kind="ExternalInput").ap()
    bias_d = nc.dram_tensor("biases", (128, 3), F32,
                            kind="ExternalInput").ap()
    zer_d = nc.dram_tensor("zer", (128, HP), xdt,
                           kind="ExternalInput").ap()
    odt = F32 if ODT_NAME == "f32r" else BF16
    out_d = nc.dram_tensor("out", (C, T, W, H), odt,
                           kind="ExternalOutput").ap()
    from contextlib import ExitStack
    with tile.TileContext(nc) as tc, ExitStack() as ctx:
        _emit(nc, tc, x_d, wkq_d, wv_d, bias_d, zer_d, out_d, ctx)
    nc.compile()
    _CACHE[key] = nc
    return nc


def run_spmd(x, wq, wk, wv, bq, bk, bv, gamma, trace=False, **kw):
    nc = _build()
    wkq, wv2, biases = _pack_weights(
        np.asarray(wq, np.float32), np.asarray(wk, np.float32),
        np.asarray(wv, np.float32), np.asarray(bq, np.float32),
        np.asarray(bk, np.float32), np.asarray(bv, np.float32),
        np.asarray(gamma, np.float32))
    x = np.asarray(x, np.float32)
    xpad = np.zeros((B, C, T, W, HP), np.float32)
    xpad[..., 1:1 + H] = x
    zer = np.zeros((128, HP), np.float32)
    if XDT_NAME == "bf16":
        xpad = _to_bf16(xpad)
        zer = _to_bf16(zer)
    in_maps = [
        {"xpad": np.ascontiguousarray(xpad[b]), "wkq": wkq, "wv2": wv2,
         "biases": biases, "zer": zer}
        for b in range(B)
    ]
    res = bass_utils.run_bass_kernel_spmd(
        nc, in_maps, core_ids=list(range(B)), trace=trace, **kw)
    out = np.stack(
        [np.asarray(res.results[b]["out"], np.float32) for b in range(B)],
        axis=0)
    return out, res


def kernel(x, wq, wk, wv, bq, bk, bv, gamma):
    out, _ = run_spmd(x, wq, wk, wv, bq, bk, bv, gamma)
    return out


# revision 34
# speedup vs baseline: 1.9139x; 1.0092x over previous
"""Trainium2 Bass kernel for conv-qkv rank-1 attention.

out = gamma * (q+bq) * sum((k+bk)*(v+bv)) + x, where q,k,v are
per-time-slice 3x3 convs (C=64 -> C=64) of x [B=8, C=64, T=16, W=64, H=64].

Sharding: data-parallel over B across 8 cores (1 example/core), conv
weights replicated. No cross-core communication.

v2 design (vs v1 baseline at ~492us):
- No bias matmul taps: bq/bv folded into the ACT/Pool evacuation bias,
  bk folded into the DVE STT op0-add scalar, gamma folded into wv/bv
  host-side. 18 PE pair-slots per (pair, block) instead of 20.
- Stationary weights in bf16: LDWEIGHTS streams half the bytes (it was
  longer than the 512-row matmul itself), moving x stays f32r.
- Host pads H to 66 so each x slice loads with ONE contiguous
  descriptor per partition (was 64 x 256B strided descriptors -> 57us
  serial startup and 205us of DMA activity).
- W-pad rows zeroed once with on-chip memzero (no zero-vector DMAs).
- Merged [128,512] out-STT (q*s+x for both slices at once), s for the
  hi slice moved 0:63 -> 64:127 with a tiny sbuf-sbuf DMA on the
  vector queue.
- v evacuated by Pool (gpsimd), q by ACT, k*v+reduce and out on DVE.
"""

import os

import numpy as np

import concourse.bacc as bacc
import concourse.bass as bass
import concourse.mybir as mybir
import concourse.tile as tile
from concourse import bass_utils

F32 = mybir.dt.float32
F32R = mybir.dt.float32r
BF16 = mybir.dt.bfloat16
ALU = mybir.AluOpType
ACTF = mybir.ActivationFunctionType

B, C, T, W, H = 8, 64, 16, 64, 64
HP = H + 2                     # host-padded H
WP = W + 2                     # SBUF-padded W rows
NPAIR = T // 2                 # slice pairs per core
RB = 8                         # W-rows per pixel block
NBLK = W // RB                 # pixel blocks per slice
BN = RB * H                    # moving free dim per matmul (512)
NTAP = 9                       # conv taps (no bias tap)

XDT_NAME = os.environ.get("BASS_XDT", "bf16")   # moving/x dtype
# walrus rejects mixed 32/16-bit matmul inputs: stationary follows moving
WDT_NAME = os.environ.get("BASS_WDT", "bf16" if XDT_NAME == "bf16" else "f32r")
# out/qs storage dtype follows x by default
ODT_NAME = os.environ.get("BASS_ODT", XDT_NAME)
# GPSIMD cannot access PSUM (BIR verifier) -> evacuations must use ACT
VEVAC = os.environ.get("BASS_VEVAC", "act")     # pool | act
# v-hi matmuls write psum partitions 64-127 of the same bank as v-lo
# (bf16 permits tile_position col 64), giving one merged v evacuation
VQUAD = os.environ.get("BASS_VQUAD", "1" if XDT_NAME == "bf16" else "0") == "1"
# Pool rejects TensorScalarPtr at codegen -> out-STT stays on DVE
POOLOUT = os.environ.get("BASS_POOLOUT", "0") == "1"


def _round22(a: np.ndarray) -> np.ndarray:
    """Round fp32 to 11 mantissa bits so the PE's FP22 read-truncation is
    exact (unbiased quantization instead of truncation)."""
    u = np.ascontiguousarray(a, np.float32).view(np.uint32).astype(np.uint64)
    u = ((u + 0x800) & 0xFFFFF000).astype(np.uint32)
    return u.view(np.float32)


def _to_bf16(a: np.ndarray) -> np.ndarray:
    import ml_dtypes
    return np.ascontiguousarray(a, np.float32).astype(ml_dtypes.bfloat16)


def _pack_w(a: np.ndarray) -> np.ndarray:
    return _to_bf16(a) if WDT_NAME == "bf16" else _round22(a)


def _pack_weights(wq, wk, wv, bq, bk, bv, gamma):
    """Pack stationary operands (no bias rows; gamma folded into wv/bv).

    wkq [128, 9, 128]: [Wk | Wq] on both partition halves (k lands on
    psum partitions 0-63 for the DVE accum op, q on 64-127).
    wv2 [128, 9, 64]: gamma*Wv on both halves (M=64).
    bias [128, 3]: col0=bq, col1=bk, col2=gamma*bv, duplicated halves.
    """
    g = float(np.asarray(gamma).reshape(-1)[0])

    def taps(w):  # [O, I, 1, 3, 3] -> [I, 9, O]
        return np.ascontiguousarray(
            w.reshape(C, C, 9).transpose(1, 2, 0), np.float32)

    wq_t, wk_t, wv_t = taps(wq), taps(wk), taps(wv) * g
    # lo chain: [Wk | Wq] (k on psum partitions 0-63); hi chain flipped
    # to [Wq | Wk] so k_{t+1} lands on partitions 64-127 and the whole
    # hi k*v/s pipeline stays on the upper partition half (no s swap)
    wkq = np.zeros((128, NTAP, 128), np.float32)
    wkq[0:64, :, 0:64] = wk_t
    wkq[0:64, :, 64:128] = wq_t
    wkq[64:128, :, 0:64] = wq_t
    wkq[64:128, :, 64:128] = wk_t

    wv2 = np.zeros((128, NTAP, 64), np.float32)
    wv2[0:64] = wv_t
    wv2[64:128] = wv_t

    bias = np.zeros((128, 3), np.float32)
    bias[0:64, 0] = bq
    bias[64:128, 0] = bq
    bias[0:64, 1] = bk
    bias[64:128, 1] = bk
    bias[0:64, 2] = bv * g
    bias[64:128, 2] = bv * g
    return _pack_w(wkq), _pack_w(wv2), bias


def _emit(nc, tc, x_d, wkq_d, wv_d, bias_d, zer_d, out_d, ctx):
    xdt = F32R if XDT_NAME == "f32r" else BF16  # storage dtype of x tiles

    const = ctx.enter_context(tc.tile_pool(name="const", bufs=1))
    state = ctx.enter_context(tc.tile_pool(name="state", bufs=1))
    # kq banks triple-buffered (6) + v shared bank double-buffered (2);
    # without VQUAD, v needs 2 banks/iter so kq drops to double-buffered
    psum = ctx.enter_context(
        tc.tile_pool(name="psum", bufs=3 if VQUAD else 2,
                     space=bass.MemorySpace.PSUM))
    psumv = ctx.enter_context(
        tc.tile_pool(name="psumv", bufs=2, space=bass.MemorySpace.PSUM))
    vpool = ctx.enter_context(tc.tile_pool(name="vpool", bufs=2))

    wdt = BF16 if WDT_NAME == "bf16" else F32R
    wkq_t = const.tile([128, NTAP, 128], wdt, tag="wkq")
    wv_t = const.tile([128, NTAP, 64], wdt, tag="wv")
    bias_t = const.tile([128, 3], F32, tag="bias")

    odt = F32 if ODT_NAME == "f32r" else BF16

    xp = [state.tile([128, WP, HP], xdt, tag=f"xp{i}", name=f"xp{i}")
          for i in range(3)]
    qs = [state.tile([128, W * H], odt, tag=f"qs{i}", name=f"qs{i}")
          for i in range(2)]
    ot = [state.tile([128, W * H], odt, tag=f"ot{i}", name=f"ot{i}")
          for i in range(2)]
    scr = state.tile([128, BN], F32, tag="scr", name="scr")
    sparts = [state.tile([128, NBLK], F32, tag=f"sp{i}", name=f"sp{i}")
              for i in range(2)]
    sfull = [state.tile([128, 1], F32, tag=f"sf{i}", name=f"sf{i}")
             for i in range(2)]

    def load_pair(p):
        t_ = xp[p % 3]
        nc.sync.dma_start(t_[0:64, 1:1 + W, :], x_d[:, 2 * p])
        nc.sync.dma_start(t_[64:128, 1:1 + W, :], x_d[:, 2 * p + 1])

    # wv first on sync (its completion sem gates the first matmul; DMA
    # completion sems lag the transfer by ~4us, so head-of-queue matters)
    nc.sync.dma_start(wv_t[:], wv_d[:])
    # zero the W-pad rows once (H-pad columns come zeroed from the host).
    # The BIR verifier rejects compute-engine writes feeding an fp32r
    # matmul, so in f32r mode the zeros come from a host tensor via DMA
    # (on the ACT queue, off the x-load path).
    for t_ in xp:
        if xdt == F32R:
            nc.scalar.dma_start(t_[:, 0, :], zer_d[:, :])
            nc.scalar.dma_start(t_[:, WP - 1, :], zer_d[:, :])
        else:
            nc.vector.memset(t_[:, 0, :], 0.0)
            nc.vector.memset(t_[:, WP - 1, :], 0.0)
    # first pair split across all three DMA-capable queues so the first
    # matmul can start after a ~2us quarter-slice load
    HW2 = W // 2
    nc.gpsimd.dma_start(xp[0][0:64, 1:1 + HW2, :], x_d[:, 0, 0:HW2])
    nc.scalar.dma_start(xp[0][0:64, 1 + HW2:1 + W, :], x_d[:, 0, HW2:W])
    nc.sync.dma_start(xp[0][64:128, 1:1 + HW2, :], x_d[:, 1, 0:HW2])
    nc.sync.dma_start(xp[0][64:128, 1 + HW2:1 + W, :], x_d[:, 1, HW2:W])
    nc.gpsimd.dma_start(wkq_t[:], wkq_d[:])
    nc.gpsimd.dma_start(bias_t[:], bias_d[:])
    # load_pair(1) is emitted inside pair 0's block loop: matmuls wait on
    # the issuing queue's DMA counter, so any DMA emitted earlier on the
    # same queue delays the first matmul

    def mm_rhs(xp_, half, tap, j):
        dy, dx = tap // 3, tap % 3
        r0 = j * RB + dy
        return xp_[64 * half:64 * half + 64, r0:r0 + RB, dx:dx + H]

    for p in range(NPAIR):
        pb = p % 2
        xp_, qs_, ot_ = xp[p % 3], qs[pb], ot[pb]

        if p + 2 < NPAIR:
            load_pair(p + 2)

        for j in range(NBLK):
            if p == 0 and j == 2:
                load_pair(1)
            if p == 0 and j == 5:
                load_pair(2)
            if VQUAD:
                v_lo = v_hi = psumv.tile([128, BN], F32, tag="v_lo",
                                         name="v_lo")
                v_hi_out = v_hi[64:128, :]
            else:
                v_lo = psumv.tile([128, BN], F32, tag="v_lo", name="v_lo")
                v_hi = psumv.tile([128, BN], F32, tag="v_hi", name="v_hi")
                v_hi_out = v_hi[0:64, :]
            kq_lo = psum.tile([128, BN], F32, tag="kq_lo")
            kq_hi = psum.tile([128, BN], F32, tag="kq_hi")

            for tap in range(NTAP):
                st, sp = tap == 0, tap == NTAP - 1
                nc.tensor.matmul(
                    v_lo[0:64, :], wv_t[0:64, tap, :],
                    mm_rhs(xp_, 0, tap, j), start=st, stop=sp)
                nc.tensor.matmul(
                    v_hi_out, wv_t[64:128, tap, :],
                    mm_rhs(xp_, 1, tap, j), start=st, stop=sp)
            for tap in range(NTAP):
                st, sp = tap == 0, tap == NTAP - 1
                nc.tensor.matmul(
                    kq_lo[:, :], wkq_t[0:64, tap, :],
                    mm_rhs(xp_, 0, tap, j), start=st, stop=sp)
                nc.tensor.matmul(
                    kq_hi[:, :], wkq_t[64:128, tap, :],
                    mm_rhs(xp_, 1, tap, j), start=st, stop=sp)

            # v + bv -> SBUF (ACT), q + bq -> SBUF (ACT; lo crosses
            # partitions 64-127 -> 0-63 to line up with x_t)
            vsb = vpool.tile([128, BN], F32, tag="vsb", name="vsb")
            if VQUAD:
                nc.scalar.activation(
                    vsb[:, :], v_lo[:, :], ACTF.Identity,
                    bias=bias_t[:, 2:3])
            else:
                nc.scalar.activation(
                    vsb[0:64, :], v_lo[0:64, :], ACTF.Identity,
                    bias=bias_t[0:64, 2:3])
                nc.scalar.activation(
                    vsb[64:128, :], v_hi[0:64, :], ACTF.Identity,
                    bias=bias_t[64:128, 2:3])
            nc.scalar.activation(
                qs_[0:64, j * BN:(j + 1) * BN], kq_lo[64:128, :],
                ACTF.Identity, bias=bias_t[64:128, 0:1])
            nc.scalar.activation(
                qs_[64:128, j * BN:(j + 1) * BN], kq_hi[0:64, :],
                ACTF.Identity, bias=bias_t[0:64, 0:1])

            # (k+bk)*v with pixel-sum accumulation; the lo chain lives on
            # partitions 0-63, the hi chain on 64-127 throughout
            nc.vector.scalar_tensor_tensor(
                out=scr[0:64, :], in0=kq_lo[0:64, :],
                scalar=bias_t[0:64, 1:2], in1=vsb[0:64, :],
                op0=ALU.add, op1=ALU.mult,
                accum_out=sparts[pb][0:64, j:j + 1])
            nc.vector.scalar_tensor_tensor(
                out=scr[64:128, :], in0=kq_hi[64:128, :],
                scalar=bias_t[64:128, 1:2], in1=vsb[64:128, :],
                op0=ALU.add, op1=ALU.mult,
                accum_out=sparts[pb][64:128, j:j + 1])

        nc.vector.tensor_reduce(
            sfull[pb][:, 0:1], sparts[pb][:, :],
            axis=mybir.AxisListType.X, op=ALU.add)

        for j in range(NBLK):
            in1 = xp_[:, 1 + j * RB:1 + (j + 1) * RB, 1:1 + H]
            if xdt == F32R:
                in1 = in1.bitcast(F32)
            # split the q*s+x epilogue between DVE and the idle Pool
            # engine (all operands in SBUF, so Pool is allowed)
            eng = nc.gpsimd if (POOLOUT and j % 2 == 1) else nc.vector
            eng.scalar_tensor_tensor(
                out=ot_[:, j * BN:(j + 1) * BN],
                in0=qs_[:, j * BN:(j + 1) * BN],
                scalar=sfull[pb][:, 0:1],
                in1=in1,
                op0=ALU.mult, op1=ALU.add)

        # out-DMAs on sync (x loads leave it mostly idle; keeping gpsimd
        # DMA-free after startup dodges its slow end-of-kernel drain)
        nc.sync.dma_start(out_d[:, 2 * p], ot_[0:64, :])
        nc.sync.dma_start(out_d[:, 2 * p + 1], ot_[64:128, :])


_CACHE = {}


def _build():
    key = (XDT_NAME, WDT_NAME, VEVAC)
    if key in _CACHE:
        return _CACHE[key]
    nc = bacc.Bacc("TRN2", target_bir_lowering=False, debug=False,
                   enable_asserts=False, num_devices=8)
    xdt = F32R if XDT_NAME == "f32r" else BF16
    wdt = BF16 if WDT_NAME == "bf16" else F32R
    x_d = nc.dram_tensor("xpad", (C, T, W, HP), xdt,
                         kind="ExternalInput").ap()
    wkq_d = nc.dram_tensor("wkq", (128, NTAP, 128), wdt,
                           kind="ExternalInput").ap()
    wv_d = nc.dram_tensor("wv2", (128, NTAP, 64), wdt,
                          kind="ExternalInput").ap()
    bias_d = nc.dram_tensor("biases", (128, 3), F32,
                            kind="ExternalInput").ap()
    zer_d = nc.dram_tensor("zer", (128, HP), xdt,
                           kind="ExternalInput").ap()
    odt = F32 if ODT_NAME == "f32r" else BF16
    out_d = nc.dram_tensor("out", (C, T, W, H), odt,
                           kind="ExternalOutput").ap()
    from contextlib import ExitStack
    with tile.TileContext(nc) as tc, ExitStack() as ctx:
        _emit(nc, tc, x_d, wkq_d, wv_d, bias_d, zer_d, out_d, ctx)
    nc.compile()
    _CACHE[key] = nc
    return nc


def run_spmd(x, wq, wk, wv, bq, bk, bv, gamma, trace=False, **kw):
    nc = _build()
    wkq, wv2, biases = _pack_weights(
        np.asarray(wq, np.float32), np.asarray(wk, np.float32),
        np.asarray(wv, np.float32), np.asarray(bq, np.float32),
        np.asarray(bk, np.float32), np.asarray(bv, np.float32),
        np.asarray(gamma, np.float32))
    x = np.asarray(x, np.float32)
    xpad = np.zeros((B, C, T, W, HP), np.float32)
    xpad[..., 1:1 + H] = x
    zer = np.zeros((128, HP), np.float32)
    if XDT_NAME == "bf16":
        xpad = _to_bf16(xpad)
        zer = _to_bf16(zer)
    in_maps = [
        {"xpad": np.ascontiguousarray(xpad[b]), "wkq": wkq, "wv2": wv2,
         "biases": biases, "zer": zer}
        for b in range(B)
    ]
    res = bass_utils.run_bass_kernel_spmd(
        nc, in_maps, core_ids=list(range(B)), trace=trace, **kw)
    out = np.stack(
        [np.asarray(res.results[b]["out"], np.float32) for b in range(B)],
        axis=0)
    return out, res


def kernel(x, wq, wk, wv, bq, bk, bv, gamma):
    out, _ = run_spmd(x, wq, wk, wv, bq, bk, bv, gamma)
    return out
